# revision 1
# baseline (speedup 1.0000x reference)
"""BRPConvEmbedding (3-layer GraphConv + AvgPool readout) on 8 Trainium2 cores.

Sharding: graphs are split contiguously across cores (32 graphs/core), so
pooling is core-local and the output is a pure concat. Each core owns the
nodes of its graphs; within a core, nodes are permuted into dst-groups of 64
nodes whose total in-degree per src-half is capped at 512 (4 chunks of 128
edge slots) via greedy bin-packing, which makes the per-group edge-chunk
layout uniform across all cores (single SPMD program).

Per layer: hn rows are fetched with SWDGE dma_gather (int16 indices; the node
table is split into two halves so indices fit in int16), the per-edge one-hot
is built on the VectorE (iota + tensor_tensor is_equal), the segment-sum runs
on the TensorE (lhsT=gathered chunk, rhs=onehot, PSUM accumulation), followed
by agg.T @ W + fused epilogue, and an AllGather of the new node features.
"""
import numpy as np
from contextlib import ExitStack

import concourse.bacc as bacc
import concourse.mybir as mybir
from concourse import tile
from concourse.bass_utils import run_bass_kernel_spmd

N_NODES = 50000
N_EDGES = 800000
D = 128
N_LAYERS = 3
N_GRAPHS = 256
NCORES = 8
GSZ = 64                  # dst nodes per group
CHUNKS_PER_HALF = 4       # 4*128 = 512 edge-slot cap per (group, half)
CAP = CHUNKS_PER_HALF * 128
GPC = N_GRAPHS // NCORES  # graphs per core


# ----------------------------------------------------------------- host prep
def _pack_groups(nodes, dA, dB):
    """Greedy bin-packing of nodes into groups of <= GSZ nodes with
    sum(dA) <= CAP and sum(dB) <= CAP per group. Returns group id per node."""
    order = np.argsort(-np.maximum(dA, dB), kind="stable")
    gids = np.full(len(nodes), -1, dtype=np.int64)
    usedA, usedB, usedN = [], [], []
    for i in order:
        a, b = dA[i], dB[i]
        best, best_fit = -1, -1.0
        for g in range(len(usedA)):
            if usedN[g] < GSZ and usedA[g] + a <= CAP and usedB[g] + b <= CAP:
                # best-fit: prefer the fullest group that still fits
                fit = max((usedA[g] + a) / CAP, (usedB[g] + b) / CAP)
                if fit > best_fit:
                    best, best_fit = g, fit
        if best < 0:
            usedA.append(0), usedB.append(0), usedN.append(0)
            best = len(usedA) - 1
        gids[i] = best
        usedA[best] += a
        usedB[best] += b
        usedN[best] += 1
    return gids, len(usedA)


def preprocess(feats, W, b, src, dst, graph_ids):
    src = np.asarray(src).astype(np.int64)
    dst = np.asarray(dst).astype(np.int64)
    graph_ids = np.asarray(graph_ids).astype(np.int64)
    feats = np.asarray(feats, dtype=np.float32)

    deg_out = np.maximum(np.bincount(src, minlength=N_NODES), 1).astype(np.float32)
    deg_in = np.maximum(np.bincount(dst, minlength=N_NODES), 1).astype(np.float32)

    node_core = graph_ids // GPC                      # node -> core
    src_half = (node_core[src] >= NCORES // 2).astype(np.int64)
    dA = np.bincount(dst[src_half == 0], minlength=N_NODES)
    dB = np.bincount(dst[src_half == 1], minlength=N_NODES)

    # pack nodes into groups per core
    core_nodes = [np.nonzero(node_core == c)[0] for c in range(NCORES)]
    packs = []
    Gmax = 0
    for c in range(NCORES):
        n = core_nodes[c]
        g, ng = _pack_groups(n, dA[n], dB[n])
        packs.append(g)
        Gmax = max(Gmax, ng)
    G = -(-Gmax // 4) * 4                             # multiple of 4 (supers of 4 groups)
    P = G // 2                                        # pairs (128-node tiles)
    NSUP = G // 4
    SH = G * GSZ                                      # rows per core shard
    R_half = (NCORES // 2) * SH
    assert R_half <= 32767, f"int16 overflow: {R_half}"

    # node -> row
    row = np.full(N_NODES, -1, dtype=np.int64)
    slot_in_group = np.zeros(N_NODES, dtype=np.int64)
    for c in range(NCORES):
        n = core_nodes[c]
        g = packs[c]
        order = np.lexsort((n, g))                    # stable by group
        n_sorted, g_sorted = n[order], g[order]
        # slot = rank within group
        slot = np.zeros(len(n), dtype=np.int64)
        _, starts = np.unique(g_sorted, return_index=True)
        for s0, s1 in zip(starts, list(starts[1:]) + [len(n)]):
            slot[s0:s1] = np.arange(s1 - s0)
        row[n_sorted] = c * SH + g_sorted * GSZ + slot
        slot_in_group[n_sorted] = slot

    # global row map for gather indices: 4 blocks (core-group x pair-half)
    # row_g(c, loc) = (c//4)*R_half + q*(R_half//2) + (c%4)*(SH//2) + (loc - q*(SH//2))
    # where q = loc >= SH//2
    loc_all = row - node_core * SH          # local row within shard (valid where row>=0)
    qh = (loc_all >= SH // 2).astype(np.int64)
    row_g = ((node_core // 4) * R_half + qh * (R_half // 2)
             + (node_core % 4) * (SH // 2) + (loc_all - qh * (SH // 2)))

    # per-core edge layout
    e_core = node_core[dst]
    e_group = np.zeros(N_EDGES, dtype=np.int64)
    for c in range(NCORES):
        m = e_core == c
        d_local = dst[m]
        lr = row[d_local] - c * SH
        e_group[m] = lr // GSZ
    e_dslot = (row[dst] % SH) % GSZ
    e_srow = row_g[src] - src_half * R_half           # int16-safe source row

    per_core = []
    for c in range(NCORES):
        m = np.nonzero(e_core == c)[0]
        g, h, sr, dslt = e_group[m], src_half[m], e_srow[m], e_dslot[m]
        order = np.lexsort((sr, h, g))
        g, h, sr, dslt = g[order], h[order], sr[order], dslt[order]
        # rank within (g, h)
        key = g * 2 + h
        rank = np.arange(len(m)) - np.searchsorted(key, key, side="left")
        k = rank // 128                               # chunk within (g,h)
        p = rank % 128
        assert (k < CHUNKS_PER_HALF).all(), "cap exceeded"
        gi = g % 4                                    # group idx in super
        s = g // 4
        c16 = gi * CHUNKS_PER_HALF + k                # chunk col within (super, half)
        j = c16 * 128 + p                             # slot within (super, half)

        # idx arrays [2*NSUP, 16, 128] (then tiled to 128 partitions)
        idx16 = np.zeros((2 * NSUP, 16, 128), dtype=np.int16)
        t = s * 2 + h
        idx16[t, j % 16, j // 16] = sr.astype(np.int16)
        idx_all = np.tile(idx16, (1, 8, 1)).reshape(2 * NSUP, 128, 128)
        idx_2d = idx_all.transpose(1, 0, 2).reshape(128, 2 * NSUP * 128).copy()

        # dst one-hot scalars [128, 2*NSUP*16], -1 for pad slots
        dstv = np.full((128, 2 * NSUP * 16), -1.0, dtype=np.float32)
        dstv[j % 128, t * 16 + c16] = dslt.astype(np.float32)

        # per-pair node scalars [128, P]
        nodes_c = core_nodes[c]
        lr = row[nodes_c] - c * SH
        deg_in_t = np.ones((128, P), dtype=np.float32)
        deg_out_t = np.ones((128, P), dtype=np.float32)
        gid_t = np.full((128, P), -1.0, dtype=np.float32)
        pr = lr // 128
        pp = lr % 128
        deg_in_t[pp, pr] = deg_in[nodes_c]
        deg_out_t[pp, pr] = deg_out[nodes_c]
        gid_t[pp, pr] = (graph_ids[nodes_c] - c * GPC).astype(np.float32)

        counts = np.maximum(
            np.bincount(graph_ids[nodes_c] - c * GPC, minlength=GPC), 1
        ).astype(np.float32).reshape(GPC, 1)

        feats_shard = np.zeros((SH, D), dtype=np.float32)
        feats_shard[lr] = feats[nodes_c]

        per_core.append(dict(
            idx=idx_2d, dstv=dstv, deg_in=deg_in_t, deg_out=deg_out_t,
            gid=gid_t, counts=counts, feats=feats_shard,
        ))

    b_rep = np.broadcast_to(
        np.asarray(b, dtype=np.float32)[None, :, :], (128, N_LAYERS, D)
    ).copy()
    meta = dict(G=G, P=P, NSUP=NSUP, SH=SH, R_half=R_half)
    shared = dict(W=np.ascontiguousarray(np.asarray(W, dtype=np.float32).transpose(1, 0, 2)),
                  b_rep=b_rep,
                  scr=np.zeros((NCORES * SH, D), dtype=np.float32))
    return per_core, shared, meta


# ------------------------------------------------------------- device build
def build(meta, rep=1, no_coll=False, no_gather=False, split_gather=2):
    G, P, NSUP, SH = meta["G"], meta["P"], meta["NSUP"], meta["SH"]
    R_half = meta["R_half"]
    CH = CHUNKS_PER_HALF
    f32 = mybir.dt.float32

    nc = bacc.Bacc("TRN2", target_bir_lowering=False, debug=False,
                   num_devices=NCORES, dynamic_dma_scratch_size=16384)

    idx_t = nc.dram_tensor("idx", [128, 2 * NSUP * 128], mybir.dt.int16, kind="ExternalInput")
    dstv_t = nc.dram_tensor("dstv", [128, 2 * NSUP * 16], f32, kind="ExternalInput")
    degi_t = nc.dram_tensor("deg_in", [128, P], f32, kind="ExternalInput")
    dego_t = nc.dram_tensor("deg_out", [128, P], f32, kind="ExternalInput")
    gid_t = nc.dram_tensor("gid", [128, P], f32, kind="ExternalInput")
    cnt_t = nc.dram_tensor("counts", [GPC, 1], f32, kind="ExternalInput")
    feats_t = nc.dram_tensor("feats", [SH, D], f32, kind="ExternalInput")
    W_t = nc.dram_tensor("W", [128, N_LAYERS, D], f32, kind="ExternalInput")
    brep_t = nc.dram_tensor("b_rep", [128, N_LAYERS, D], f32, kind="ExternalInput")
    scr_t = [nc.dram_tensor(f"scr{i}", [NCORES * SH, D], f32, kind="ExternalInput")
             for i in range(2)]
    out_t = nc.dram_tensor("out", [GPC, D], f32, kind="ExternalOutput")

    HSH = SH // 2
    hn_part = [
        [nc.dram_tensor(f"hn_p{i}_{q}", [NCORES * HSH, D], f32,
                        kind="Internal", addr_space="Shared") for q in range(2)]
        for i in range(N_LAYERS)
    ]

    def ag_and_copy(nc, hn_shard, l):
        """AllGather hn_shard (split by pair-halves) into scr[l % 2]."""
        scr = scr_t[l % 2]
        for q in range(2):
            nc.gpsimd.collective_compute(
                "AllGather", mybir.AluOpType.bypass,
                replica_groups=[list(range(NCORES))],
                ins=[hn_shard[q * HSH:(q + 1) * HSH, :].opt()],
                outs=[hn_part[l][q].ap().opt()],
            )
            half_blk = (NCORES // 2) * HSH
            for cg in range(2):
                dst0 = cg * (NCORES // 2) * SH + q * half_blk
                nc.sync.dma_start(
                    scr.ap()[dst0:dst0 + half_blk, :],
                    hn_part[l][q].ap()[cg * half_blk:(cg + 1) * half_blk, :],
                )

    with tile.TileContext(nc) as tc, ExitStack() as ctx:
        dram = ctx.enter_context(tc.tile_pool(name="dram", bufs=1, space="DRAM"))
        stat = ctx.enter_context(tc.tile_pool(name="stat", bufs=1))
        gpool = ctx.enter_context(tc.tile_pool(name="gath", bufs=4))
        opool = ctx.enter_context(tc.tile_pool(name="oh", bufs=4))
        spool = ctx.enter_context(tc.tile_pool(name="sb", bufs=4))
        ppool = ctx.enter_context(tc.tile_pool(name="agg_ps", bufs=4, space="PSUM"))
        hpool = ctx.enter_context(tc.tile_pool(name="h_ps", bufs=2, space="PSUM"))
        plpool = ctx.enter_context(tc.tile_pool(name="pool_ps", bufs=1, space="PSUM"))

        hn_shard = dram.tile([SH, D], f32)

        # ---- statics
        idx_sb = stat.tile([128, 2 * NSUP * 128], mybir.dt.int16)
        nc.sync.dma_start(idx_sb[:], idx_t.ap())
        dstv_sb = stat.tile([128, 2 * NSUP * 16], f32)
        nc.sync.dma_start(dstv_sb[:], dstv_t.ap())
        W_sb = stat.tile([128, N_LAYERS, D], f32)
        nc.sync.dma_start(W_sb[:], W_t.ap())
        brep_sb = stat.tile([128, N_LAYERS, D], f32)
        nc.sync.dma_start(brep_sb[:], brep_t.ap())
        gid_sb = stat.tile([128, P], f32)
        nc.sync.dma_start(gid_sb[:], gid_t.ap())
        cnt_sb = stat.tile([GPC, 1], f32)
        nc.sync.dma_start(cnt_sb[:], cnt_t.ap())

        degi_sb = stat.tile([128, P], f32)
        nc.sync.dma_start(degi_sb[:], degi_t.ap())
        dego_sb = stat.tile([128, P], f32)
        nc.sync.dma_start(dego_sb[:], dego_t.ap())
        ni_sb = stat.tile([128, P], f32)   # rsqrt(deg_in)
        no_sb = stat.tile([128, P], f32)   # rsqrt(deg_out)
        nc.vector.reciprocal(ni_sb[:], degi_sb[:])
        nc.scalar.activation(ni_sb[:], ni_sb[:], mybir.ActivationFunctionType.Sqrt)
        nc.vector.reciprocal(no_sb[:], dego_sb[:])
        nc.scalar.activation(no_sb[:], no_sb[:], mybir.ActivationFunctionType.Sqrt)
        rc_sb = stat.tile([GPC, 1], f32)   # 1/counts
        nc.vector.reciprocal(rc_sb[:], cnt_sb[:])

        iota16 = stat.tile([128, GSZ], mybir.dt.int16)
        nc.gpsimd.iota(iota16[:], pattern=[[1, GSZ]], base=0, channel_multiplier=0)
        iota_f = stat.tile([128, GSZ], f32)
        nc.vector.tensor_copy(iota_f[:], iota16[:])

        # graph one-hot [128, P, GPC] (built once; pooling uses layer-2 h)
        groh = stat.tile([128, P, GPC], f32)
        nc.vector.tensor_tensor(
            out=groh[:],
            in0=iota_f[:, :GPC].unsqueeze(1).broadcast_to([128, P, GPC]),
            in1=gid_sb[:].unsqueeze(2).broadcast_to([128, P, GPC]),
            op=mybir.AluOpType.is_equal,
        )

        for _ in range(rep):
            # ---- layer 0 input: hn0 = feats * norm_out
            for pr in range(P):
                ft = spool.tile([128, D], f32, tag="ft")
                nc.sync.dma_start(ft[:], feats_t.ap()[pr * 128:(pr + 1) * 128, :])
                hn0 = spool.tile([128, D], f32, tag="hn")
                nc.vector.tensor_scalar_mul(hn0[:], ft[:], no_sb[:, pr:pr + 1])
                nc.sync.dma_start(hn_shard[pr * 128:(pr + 1) * 128, :], hn0[:])
            if not no_coll:
                ag_and_copy(nc, hn_shard, 0)

            pool_ps = plpool.tile([GPC, D], f32)

            for l in range(N_LAYERS):
                for s in range(NSUP):
                    gA = gpool.tile([128, 4 * CH, D], f32, tag="gA")
                    gB = gpool.tile([128, 4 * CH, D], f32, tag="gB")
                    if split_gather > 1:
                        NP = split_gather          # pieces per half
                        NH = 4 * CH * 128 // NP
                        CPP = 4 * CH // NP         # chunks per piece
                        SCOL = 128 // NP           # idx cols per piece
                        for hh, gt in ((0, gA), (1, gB)):
                            base = scr_t[l % 2].ap()[0:R_half, :] if hh == 0 \
                                else scr_t[l % 2].ap()[R_half:, :]
                            for piece in range(NP):
                                nc.gpsimd.dma_gather(
                                    out_ap=gt[:, piece * CPP:(piece + 1) * CPP, :],
                                    in_ap=base,
                                    idxs_ap=idx_sb[:, (2 * s + hh) * 128 + piece * SCOL:
                                                   (2 * s + hh) * 128 + (piece + 1) * SCOL],
                                    num_idxs=NH, num_idxs_reg=NH,
                                    elem_size=D, single_packet=False)
                    elif not no_gather:
                        nc.gpsimd.dma_gather(
                            out_ap=gA[:], in_ap=scr_t[l % 2].ap()[0:R_half, :],
                            idxs_ap=idx_sb[:, (2 * s) * 128:(2 * s + 1) * 128],
                            num_idxs=4 * CH * 128, num_idxs_reg=4 * CH * 128,
                            elem_size=D, single_packet=False,
                        )
                        nc.gpsimd.dma_gather(
                            out_ap=gB[:], in_ap=scr_t[l % 2].ap()[R_half:, :],
                            idxs_ap=idx_sb[:, (2 * s + 1) * 128:(2 * s + 2) * 128],
                            num_idxs=4 * CH * 128, num_idxs_reg=4 * CH * 128,
                            elem_size=D, single_packet=False,
                        )
                    ohA = opool.tile([128, 4 * CH, GSZ], f32, tag="ohA")
                    ohB = opool.tile([128, 4 * CH, GSZ], f32, tag="ohB")
                    nc.vector.tensor_tensor(
                        out=ohA[:],
                        in0=iota_f[:].unsqueeze(1).broadcast_to([128, 4 * CH, GSZ]),
                        in1=dstv_sb[:, (2 * s) * 16:(2 * s) * 16 + 16]
                            .unsqueeze(2).broadcast_to([128, 4 * CH, GSZ]),
                        op=mybir.AluOpType.is_equal,
                    )
                    nc.vector.tensor_tensor(
                        out=ohB[:],
                        in0=iota_f[:].unsqueeze(1).broadcast_to([128, 4 * CH, GSZ]),
                        in1=dstv_sb[:, (2 * s + 1) * 16:(2 * s + 1) * 16 + 16]
                            .unsqueeze(2).broadcast_to([128, 4 * CH, GSZ]),
                        op=mybir.AluOpType.is_equal,
                    )
                    for pi in range(2):         # pairs in super
                        pr = s * 2 + pi
                        agg = ppool.tile([128, 128], f32, tag="agg")
                        for gj in range(2):     # groups in pair
                            gi = pi * 2 + gj
                            off = gj * GSZ
                            for k in range(CH):
                                nc.tensor.matmul(
                                    agg[:, off:off + GSZ],
                                    gA[:, gi * CH + k, :],
                                    ohA[:, gi * CH + k, :],
                                    start=(k == 0), stop=False,
                                    skip_group_check=True,
                                )
                            for k in range(CH):
                                nc.tensor.matmul(
                                    agg[:, off:off + GSZ],
                                    gB[:, gi * CH + k, :],
                                    ohB[:, gi * CH + k, :],
                                    start=False, stop=(k == CH - 1),
                                    skip_group_check=True,
                                )
                        agg_sb = spool.tile([128, 128], f32, tag="aggsb")
                        nc.scalar.copy(agg_sb[:], agg[:])
                        hps = hpool.tile([128, D], f32, tag="hps")
                        nc.tensor.matmul(hps[:], agg_sb[:], W_sb[:, l, :],
                                         start=True, stop=True)
                        t_sb = spool.tile([128, D], f32, tag="tsb")
                        nc.vector.scalar_tensor_tensor(
                            out=t_sb[:], in0=hps[:], scalar=ni_sb[:, pr:pr + 1],
                            in1=brep_sb[:, l, :],
                            op0=mybir.AluOpType.mult, op1=mybir.AluOpType.add,
                        )
                        if l < N_LAYERS - 1:
                            hn = spool.tile([128, D], f32, tag="hn2")
                            nc.vector.tensor_scalar(
                                out=hn[:], in0=t_sb[:],
                                scalar1=0.0, scalar2=no_sb[:, pr:pr + 1],
                                op0=mybir.AluOpType.max, op1=mybir.AluOpType.mult,
                            )
                            nc.sync.dma_start(
                                hn_shard[pr * 128:(pr + 1) * 128, :], hn[:])
                        else:
                            h_sb = spool.tile([128, D], f32, tag="hsb")
                            nc.vector.tensor_scalar_max(h_sb[:], t_sb[:], 0.0)
                            nc.tensor.matmul(
                                pool_ps[:], groh[:, pr, :], h_sb[:],
                                start=(pr == 0), stop=(pr == P - 1),
                            )
                if l < N_LAYERS - 1 and not no_coll:
                    ag_and_copy(nc, hn_shard, l + 1)

            pool_sb = spool.tile([GPC, D], f32, tag="poolsb")
            nc.vector.tensor_scalar_mul(pool_sb[:], pool_ps[:], rc_sb[:])
            nc.sync.dma_start(out_t.ap(), pool_sb[:])

    nc.compile()
    return nc


def make_in_maps(per_core, shared):
    in_maps = []
    for c in range(NCORES):
        pc = per_core[c]
        in_maps.append({
            "idx": pc["idx"], "dstv": pc["dstv"], "deg_in": pc["deg_in"],
            "deg_out": pc["deg_out"], "gid": pc["gid"], "counts": pc["counts"],
            "feats": pc["feats"], "W": shared["W"], "b_rep": shared["b_rep"],
            "scr0": shared["scr"], "scr1": shared["scr"],
        })
    return in_maps


def kernel(**inputs) -> np.ndarray:
    per_core, shared, meta = preprocess(**inputs)
    nc = build(meta, rep=1)
    in_maps = make_in_maps(per_core, shared)
    res = run_bass_kernel_spmd(nc, in_maps, core_ids=list(range(NCORES)))
    return np.concatenate([res.results[c]["out"] for c in range(NCORES)], axis=0)



# revision 4
# speedup vs baseline: 2.7671x; 2.7671x over previous
"""BRPConvEmbedding (3-layer GraphConv + AvgPool readout) on 8 Trainium2 cores.

Sharding: graphs split contiguously across cores (32 graphs/core); each core
owns its graphs' nodes, so pooling is core-local and the output is a concat.
Within a core, nodes are pre-committed to two halves (A/B) and each half is
packed into dst-groups of <=64 nodes whose in-degree per source-half is capped
at 512 (4 chunks of 128 edge slots), giving a uniform SPMD layout.

Per layer the full node-feature table lives in two bf16 halves (all cores'
A-rows / B-rows), built by one AllGather each; layer 0's halves are
host-precomputed (feats * norm_out) and passed in, so layer 0 needs no
collective. SpMM: per-edge rows are fetched with SWDGE dma_gather (int16
indices, 4 parallel queues), the per-edge one-hot is built on the VectorE, and
the segment-sum runs on the TensorE via gathered-chunk x one-hot matmuls in
bf16 with fp32 PSUM accumulation. The A-half runs for all supers first (so the
B-half AllGather overlaps compute); each pair's A/B partial aggregates are
evicted to SBUF and combined by two accumulating W-matmuls, then the epilogue
(x norm_in, +b, relu, x norm_out) writes bf16 rows for the next AllGather.
"""
import numpy as np
from contextlib import ExitStack

import concourse.bacc as bacc
import concourse.mybir as mybir
from concourse import tile
from concourse.bass_utils import run_bass_kernel_spmd

N_NODES = 50000
N_EDGES = 800000
D = 128
N_LAYERS = 3
N_GRAPHS = 256
NCORES = 8
GSZ = 64                  # dst nodes per group
CHUNKS_PER_HALF = 4       # 4*128 = 512 edge-slot cap per (group, src-half)
CAP = CHUNKS_PER_HALF * 128
GPC = N_GRAPHS // NCORES  # graphs per core
NQ = 4                    # SWDGE queues
PIECES = 4                # gather instructions per (super, half)


# ----------------------------------------------------------------- host prep
def _pack_groups(dA, dB):
    """Greedy bin-packing: nodes (rows of dA/dB) into groups of <= GSZ nodes
    with sum(dA) <= CAP and sum(dB) <= CAP. Returns group ids."""
    n = len(dA)
    order = np.argsort(-np.maximum(dA, dB), kind="stable")
    gids = np.full(n, -1, dtype=np.int64)
    usedA, usedB, usedN = [], [], []
    for i in order:
        a, b = dA[i], dB[i]
        best, best_fit = -1, -1.0
        for g in range(len(usedA)):
            if usedN[g] < GSZ and usedA[g] + a <= CAP and usedB[g] + b <= CAP:
                fit = max((usedA[g] + a) / CAP, (usedB[g] + b) / CAP)
                if fit > best_fit:
                    best, best_fit = g, fit
        if best < 0:
            usedA.append(0), usedB.append(0), usedN.append(0)
            best = len(usedA) - 1
        gids[i] = best
        usedA[best] += a
        usedB[best] += b
        usedN[best] += 1
    return gids, len(usedA)


def _to_bf16(x):
    import jax.numpy as jnp
    return np.asarray(jnp.asarray(np.asarray(x, np.float32), dtype=jnp.bfloat16))


def preprocess(feats, W, b, src, dst, graph_ids):
    src = np.asarray(src).astype(np.int64)
    dst = np.asarray(dst).astype(np.int64)
    graph_ids = np.asarray(graph_ids).astype(np.int64)
    feats = np.asarray(feats, dtype=np.float32)

    deg_out = np.maximum(np.bincount(src, minlength=N_NODES), 1).astype(np.float32)
    deg_in = np.maximum(np.bincount(dst, minlength=N_NODES), 1).astype(np.float32)
    node_core = graph_ids // GPC

    # pre-commit each node to half A(0)/B(1): per core, alternate over nodes
    # sorted by out-degree so both the source split and node counts balance
    half = np.zeros(N_NODES, dtype=np.int64)
    for c in range(NCORES):
        n = np.nonzero(node_core == c)[0]
        order = n[np.argsort(-deg_out[n], kind="stable")]
        half[order[1::2]] = 1

    src_half = half[src]
    dA = np.bincount(dst[src_half == 0], minlength=N_NODES)
    dB = np.bincount(dst[src_half == 1], minlength=N_NODES)

    # pack each (core, half) separately
    packs = {}
    Ghalf = 0
    for c in range(NCORES):
        for hh in range(2):
            n = np.nonzero((node_core == c) & (half == hh))[0]
            g, ng = _pack_groups(dA[n], dB[n])
            packs[(c, hh)] = (n, g)
            Ghalf = max(Ghalf, ng)
    Ghalf = -(-Ghalf // 4) * 4            # multiple of 4: NSUP even, pair
                                          # P/2 boundary on a super boundary
    G = 2 * Ghalf
    P = G // 2                            # pairs (128-node tiles)
    NSUP = G // 4
    SH = G * GSZ                          # rows per core shard
    HSH = SH // 2
    RT = NCORES * HSH                     # rows per half-table
    assert RT <= 32767, f"int16 overflow: {RT}"

    # node -> loc (row within core shard)
    loc = np.full(N_NODES, -1, dtype=np.int64)
    for c in range(NCORES):
        for hh in range(2):
            n, g = packs[(c, hh)]
            order = np.lexsort((n, g))
            n_s, g_s = n[order], g[order]
            slot = np.zeros(len(n), dtype=np.int64)
            _, starts = np.unique(g_s, return_index=True)
            for s0, s1 in zip(starts, list(starts[1:]) + [len(n)]):
                slot[s0:s1] = np.arange(s1 - s0)
            loc[n_s] = (hh * Ghalf + g_s) * GSZ + slot

    srow = node_core * HSH + (loc % HSH)  # row within half-table

    # per-core edge layout
    e_core = node_core[dst]
    e_group = loc[dst] // GSZ
    e_dslot = loc[dst] % GSZ
    e_srow = srow[src]

    per_core = []
    for c in range(NCORES):
        m = np.nonzero(e_core == c)[0]
        g, h, sr, dslt = e_group[m], src_half[m], e_srow[m], e_dslot[m]
        order = np.lexsort((sr, h, g))
        g, h, sr, dslt = g[order], h[order], sr[order], dslt[order]
        key = g * 2 + h
        rank = np.arange(len(m)) - np.searchsorted(key, key, side="left")
        k = rank // 128
        p = rank % 128
        assert (k < CHUNKS_PER_HALF).all(), "cap exceeded"
        s = g // 4
        gi = g % 4
        c16 = gi * CHUNKS_PER_HALF + k        # chunk col within (super, half)
        j = c16 * 128 + p                     # slot within (super, half)
        t = s * 2 + h

        idx16 = np.zeros((2 * NSUP, 16, 128), dtype=np.int16)
        idx16[t, j % 16, j // 16] = sr.astype(np.int16)
        idx_all = np.tile(idx16, (1, 8, 1)).reshape(2 * NSUP, 128, 128)
        idx_2d = idx_all.transpose(1, 0, 2).reshape(128, 2 * NSUP * 128).copy()

        dstv = np.full((128, 2 * NSUP * 16), -1.0, dtype=np.float32)
        dstv[j % 128, t * 16 + c16] = dslt.astype(np.float32)

        # per-pair node scalars [128, P]
        nodes_c = np.nonzero(node_core == c)[0]
        lr = loc[nodes_c]
        ni_t = np.ones((128, P), dtype=np.float32)
        no_t = np.ones((128, P), dtype=np.float32)
        gid_t = np.full((128, P), -1, dtype=np.int64)
        pr = lr // 128
        pp = lr % 128
        ni_t[pp, pr] = 1.0 / np.sqrt(deg_in[nodes_c])
        no_t[pp, pr] = 1.0 / np.sqrt(deg_out[nodes_c])
        gid_t[pp, pr] = graph_ids[nodes_c] - c * GPC
        groh = np.zeros((128, P, GPC), dtype=np.float32)
        pg, prr = np.nonzero(gid_t >= 0)
        groh[pg, prr, gid_t[pg, prr]] = 1.0

        rc = (1.0 / np.maximum(
            np.bincount(graph_ids[nodes_c] - c * GPC, minlength=GPC), 1
        )).astype(np.float32).reshape(GPC, 1)

        per_core.append(dict(
            idx=idx_2d, dstv=_to_bf16(dstv), ni=ni_t, no=no_t,
            groh=_to_bf16(groh), rc=rc,
        ))

    # layer-0 half-tables: hn0 = feats * norm_out, bf16, in AllGather layout
    hn0 = feats * (1.0 / np.sqrt(deg_out))[:, None]
    t0 = np.zeros((2, RT, D), dtype=np.float32)
    nodes = np.nonzero(loc >= 0)[0]
    t0[half[nodes], srow[nodes]] = hn0[nodes]
    table0A = _to_bf16(t0[0])
    table0B = _to_bf16(t0[1])

    b_rep = np.broadcast_to(
        np.asarray(b, dtype=np.float32)[None, :, :], (128, N_LAYERS, D)
    ).copy()
    W_t = _to_bf16(np.ascontiguousarray(
        np.asarray(W, dtype=np.float32).transpose(1, 0, 2)))

    meta = dict(G=G, P=P, NSUP=NSUP, SH=SH, HSH=HSH, RT=RT)
    shared = dict(W=W_t, b_rep=b_rep, t0A=table0A, t0B=table0B)
    return per_core, shared, meta


# ------------------------------------------------------------- device build
def build(meta, rep=1, no_coll=False):
    G, P, NSUP, SH, HSH, RT = (meta[k] for k in ("G", "P", "NSUP", "SH", "HSH", "RT"))
    CH = CHUNKS_PER_HALF
    f32 = mybir.dt.float32
    bf16 = mybir.dt.bfloat16

    nc = bacc.Bacc("TRN2", target_bir_lowering=False, debug=False,
                   num_devices=NCORES, dynamic_dma_scratch_size=16384,
                   num_swdge_queues=NQ)

    idx_t = nc.dram_tensor("idx", [128, 2 * NSUP * 128], mybir.dt.int16, kind="ExternalInput")
    dstv_t = nc.dram_tensor("dstv", [128, 2 * NSUP * 16], bf16, kind="ExternalInput")
    ni_t = nc.dram_tensor("ni", [128, P], f32, kind="ExternalInput")
    no_t = nc.dram_tensor("no", [128, P], f32, kind="ExternalInput")
    groh_t = nc.dram_tensor("groh", [128, P, GPC], bf16, kind="ExternalInput")
    rc_t = nc.dram_tensor("rc", [GPC, 1], f32, kind="ExternalInput")
    t0A_t = nc.dram_tensor("t0A", [RT, D], bf16, kind="ExternalInput")
    t0B_t = nc.dram_tensor("t0B", [RT, D], bf16, kind="ExternalInput")
    W_t = nc.dram_tensor("W", [128, N_LAYERS, D], bf16, kind="ExternalInput")
    brep_t = nc.dram_tensor("b_rep", [128, N_LAYERS, D], f32, kind="ExternalInput")
    out_t = nc.dram_tensor("out", [GPC, D], f32, kind="ExternalOutput")

    # AllGather outputs for layers 1, 2: [half][RT, D]
    ag = [[nc.dram_tensor(f"ag{l}_{q}", [RT, D], bf16,
                          kind="Internal", addr_space="Shared") for q in range(2)]
          for l in range(1, N_LAYERS)]

    SCOL = (2048 // PIECES) // 16          # idx cols per gather piece
    CPP = (CH * 4) // PIECES               # chunks per gather piece

    with tile.TileContext(nc) as tc, ExitStack() as ctx:
        dram = ctx.enter_context(tc.tile_pool(name="dram", bufs=1, space="DRAM"))
        stat = ctx.enter_context(tc.tile_pool(name="stat", bufs=1))
        gpool = ctx.enter_context(tc.tile_pool(name="gath", bufs=4))
        opool = ctx.enter_context(tc.tile_pool(name="oh", bufs=4))
        spool = ctx.enter_context(tc.tile_pool(name="sb", bufs=4))
        ppool = ctx.enter_context(tc.tile_pool(name="agg_ps", bufs=4, space="PSUM"))
        hpool = ctx.enter_context(tc.tile_pool(name="h_ps", bufs=2, space="PSUM"))
        plpool = ctx.enter_context(tc.tile_pool(name="pool_ps", bufs=1, space="PSUM"))

        hn_shard = dram.tile([SH, D], bf16)

        # ---- statics
        idx_sb = stat.tile([128, 2 * NSUP * 128], mybir.dt.int16)
        nc.sync.dma_start(idx_sb[:], idx_t.ap())
        dstv_sb = stat.tile([128, 2 * NSUP * 16], bf16)
        nc.sync.dma_start(dstv_sb[:], dstv_t.ap())
        W_sb = stat.tile([128, N_LAYERS, D], bf16)
        nc.sync.dma_start(W_sb[:], W_t.ap())
        brep_sb = stat.tile([128, N_LAYERS, D], f32)
        nc.sync.dma_start(brep_sb[:], brep_t.ap())
        groh_sb = stat.tile([128, P, GPC], bf16)
        nc.sync.dma_start(groh_sb[:], groh_t.ap())
        ni_sb = stat.tile([128, P], f32)
        nc.sync.dma_start(ni_sb[:], ni_t.ap())
        no_sb = stat.tile([128, P], f32)
        nc.sync.dma_start(no_sb[:], no_t.ap())
        rc_sb = stat.tile([GPC, 1], f32)
        nc.sync.dma_start(rc_sb[:], rc_t.ap())

        iota16 = stat.tile([128, GSZ], mybir.dt.int16)
        nc.gpsimd.iota(iota16[:], pattern=[[1, GSZ]], base=0, channel_multiplier=0)
        iota_f = stat.tile([128, GSZ], bf16)
        nc.vector.tensor_copy(iota_f[:], iota16[:])

        aggA_sb = stat.tile([128, P, 128], bf16)
        aggB_sb = stat.tile([128, P, 128], bf16)

        def gather_half(gt, src_ap, s, hh):
            base_col = (2 * s + hh) * 128
            for piece in range(PIECES):
                nc.gpsimd.dma_gather(
                    out_ap=gt[:, piece * CPP:(piece + 1) * CPP, :],
                    in_ap=src_ap,
                    idxs_ap=idx_sb[:, base_col + piece * SCOL:
                                   base_col + (piece + 1) * SCOL],
                    num_idxs=2048 // PIECES, num_idxs_reg=2048 // PIECES,
                    elem_size=D, single_packet=False,
                    queue_num=piece % NQ,
                )

        def build_oh(s, hh, tag):
            oh = opool.tile([128, 4 * CH, GSZ], bf16, tag=tag)
            t16 = (2 * s + hh) * 16
            nc.vector.tensor_tensor(
                out=oh[:],
                in0=iota_f[:].unsqueeze(1).broadcast_to([128, 4 * CH, GSZ]),
                in1=dstv_sb[:, t16:t16 + 16]
                    .unsqueeze(2).broadcast_to([128, 4 * CH, GSZ]),
                op=mybir.AluOpType.is_equal,
            )
            return oh

        def seg_matmuls(gt, oh, s, agg_dst):
            """8 matmuls per pair accumulating [f, dslot] into agg PSUM,
            then evict to agg_dst[:, pr, :] (bf16)."""
            for pi in range(2):
                pr = s * 2 + pi
                agg = ppool.tile([128, 128], f32, tag="agg")
                for gj in range(2):
                    gi = pi * 2 + gj
                    off = gj * GSZ
                    for k in range(CH):
                        nc.tensor.matmul(
                            agg[:, off:off + GSZ],
                            gt[:, gi * CH + k, :],
                            oh[:, gi * CH + k, :],
                            start=(k == 0), stop=(k == CH - 1 and gj == 1),
                            skip_group_check=True,
                        )
                nc.scalar.copy(agg_dst[:, pr, :], agg[:])

        for _ in range(rep):
            pool_ps = plpool.tile([GPC, D], f32)

            for l in range(N_LAYERS):
                if l == 0:
                    srcA, srcB = t0A_t.ap(), t0B_t.ap()
                else:
                    srcA, srcB = ag[l - 1][0].ap(), ag[l - 1][1].ap()

                # phase A: all supers' A-half work
                for s in range(NSUP):
                    gA = gpool.tile([128, 4 * CH, D], bf16, tag="gA")
                    gather_half(gA, srcA, s, 0)
                    ohA = build_oh(s, 0, "ohA")
                    seg_matmuls(gA, ohA, s, aggA_sb)

                # phase B + dense + epilogue
                for s in range(NSUP):
                    gB = gpool.tile([128, 4 * CH, D], bf16, tag="gB")
                    gather_half(gB, srcB, s, 1)
                    ohB = build_oh(s, 1, "ohB")
                    seg_matmuls(gB, ohB, s, aggB_sb)
                    for pi in range(2):
                        pr = s * 2 + pi
                        hps = hpool.tile([128, D], f32, tag="hps")
                        nc.tensor.matmul(hps[:], aggA_sb[:, pr, :], W_sb[:, l, :],
                                         start=True, stop=False,
                                         skip_group_check=True)
                        nc.tensor.matmul(hps[:], aggB_sb[:, pr, :], W_sb[:, l, :],
                                         start=False, stop=True,
                                         skip_group_check=True)
                        t_sb = spool.tile([128, D], f32, tag="tsb")
                        nc.vector.scalar_tensor_tensor(
                            out=t_sb[:], in0=hps[:], scalar=ni_sb[:, pr:pr + 1],
                            in1=brep_sb[:, l, :],
                            op0=mybir.AluOpType.mult, op1=mybir.AluOpType.add,
                        )
                        if l < N_LAYERS - 1:
                            hn = spool.tile([128, D], bf16, tag="hn")
                            nc.vector.tensor_scalar(
                                out=hn[:], in0=t_sb[:],
                                scalar1=0.0, scalar2=no_sb[:, pr:pr + 1],
                                op0=mybir.AluOpType.max, op1=mybir.AluOpType.mult,
                            )
                            nc.sync.dma_start(
                                hn_shard[pr * 128:(pr + 1) * 128, :], hn[:])
                        else:
                            h_sb = spool.tile([128, D], bf16, tag="hsb")
                            nc.vector.tensor_scalar_max(h_sb[:], t_sb[:], 0.0)
                            nc.tensor.matmul(
                                pool_ps[:], groh_sb[:, pr, :], h_sb[:],
                                start=(pr == 0), stop=(pr == P - 1),
                                skip_group_check=True,
                            )
                    # fire next layer's AllGathers as soon as their input
                    # half is fully written
                    if l < N_LAYERS - 1 and not no_coll:
                        if s == NSUP // 2 - 1:        # pairs 0..P/2-1 done
                            nc.gpsimd.collective_compute(
                                "AllGather", mybir.AluOpType.bypass,
                                replica_groups=[list(range(NCORES))],
                                ins=[hn_shard[0:HSH, :].opt()],
                                outs=[ag[l][0].ap().opt()],
                            )
                        if s == NSUP - 1:
                            nc.gpsimd.collective_compute(
                                "AllGather", mybir.AluOpType.bypass,
                                replica_groups=[list(range(NCORES))],
                                ins=[hn_shard[HSH:SH, :].opt()],
                                outs=[ag[l][1].ap().opt()],
                            )

            pool_sb = spool.tile([GPC, D], f32, tag="poolsb")
            nc.vector.tensor_scalar_mul(pool_sb[:], pool_ps[:], rc_sb[:])
            nc.sync.dma_start(out_t.ap(), pool_sb[:])

    nc.compile()
    return nc


def make_in_maps(per_core, shared):
    in_maps = []
    for c in range(NCORES):
        pc = per_core[c]
        in_maps.append({
            "idx": pc["idx"], "dstv": pc["dstv"], "ni": pc["ni"],
            "no": pc["no"], "groh": pc["groh"], "rc": pc["rc"],
            "t0A": shared["t0A"], "t0B": shared["t0B"],
            "W": shared["W"], "b_rep": shared["b_rep"],
        })
    return in_maps


def kernel(**inputs) -> np.ndarray:
    per_core, shared, meta = preprocess(**inputs)
    nc = build(meta, rep=1)
    in_maps = make_in_maps(per_core, shared)
    res = run_bass_kernel_spmd(nc, in_maps, core_ids=list(range(NCORES)))
    return np.concatenate([res.results[c]["out"] for c in range(NCORES)], axis=0)


# revision 12
# speedup vs baseline: 4.6184x; 1.6690x over previous
"""BRPConvEmbedding (3-layer GraphConv + AvgPool readout) on 8 Trainium2 cores.

Sharding: graphs split contiguously across cores (32 graphs/core); each core
owns its graphs' nodes, so pooling is core-local and the output is a concat.
Within a core, nodes are pre-committed to two halves (A/B) and each half is
packed into dst-groups of <=64 nodes whose in-degree per source-half is capped
at 512 (4 chunks of 128 edge slots), giving a uniform SPMD layout.

Per layer the full node-feature table lives in two bf16 halves (all cores'
A-rows / B-rows), built by one AllGather each; layer 0's halves are
host-precomputed (feats * norm_out) and passed in, so layer 0 needs no
collective. SpMM: per-edge rows are fetched with SWDGE dma_gather (int16
indices, 4 parallel queues), the per-edge one-hot is built on the VectorE, and
the segment-sum runs on the TensorE via gathered-chunk x one-hot matmuls in
bf16 with fp32 PSUM accumulation. The A-half runs for all supers first (so the
B-half AllGather overlaps compute); each pair's A/B partial aggregates are
evicted to SBUF and combined by two accumulating W-matmuls, then the epilogue
(x norm_in, +b, relu, x norm_out) writes bf16 rows for the next AllGather.
"""
import numpy as np
from contextlib import ExitStack

import concourse.bacc as bacc
import concourse.mybir as mybir
from concourse import tile
from concourse.bass_utils import run_bass_kernel_spmd

N_NODES = 50000
N_EDGES = 800000
D = 128
N_LAYERS = 3
N_GRAPHS = 256
NCORES = 8
GSZ = 64                  # dst nodes per group
CHUNKS_PER_HALF = 4       # 4*128 = 512 edge-slot cap per (group, src-half)
CAP = CHUNKS_PER_HALF * 128
GPC = N_GRAPHS // NCORES  # graphs per core
NQ = 4                    # SWDGE queues
PIECES = 4                # gather instructions per (super, half)


# ----------------------------------------------------------------- host prep
def _pack_groups(dA, dB):
    """Greedy bin-packing: nodes (rows of dA/dB) into groups of <= GSZ nodes
    with sum(dA) <= CAP and sum(dB) <= CAP. Returns group ids."""
    n = len(dA)
    order = np.argsort(-np.maximum(dA, dB), kind="stable")
    gids = np.full(n, -1, dtype=np.int64)
    usedA, usedB, usedN = [], [], []
    for i in order:
        a, b = dA[i], dB[i]
        best, best_fit = -1, -1.0
        for g in range(len(usedA)):
            if usedN[g] < GSZ and usedA[g] + a <= CAP and usedB[g] + b <= CAP:
                fit = max((usedA[g] + a) / CAP, (usedB[g] + b) / CAP)
                if fit > best_fit:
                    best, best_fit = g, fit
        if best < 0:
            usedA.append(0), usedB.append(0), usedN.append(0)
            best = len(usedA) - 1
        gids[i] = best
        usedA[best] += a
        usedB[best] += b
        usedN[best] += 1
    return gids, len(usedA)


def _to_bf16(x):
    import jax.numpy as jnp
    return np.asarray(jnp.asarray(np.asarray(x, np.float32), dtype=jnp.bfloat16))


def preprocess(feats, W, b, src, dst, graph_ids):
    src = np.asarray(src).astype(np.int64)
    dst = np.asarray(dst).astype(np.int64)
    graph_ids = np.asarray(graph_ids).astype(np.int64)
    feats = np.asarray(feats, dtype=np.float32)

    deg_out = np.maximum(np.bincount(src, minlength=N_NODES), 1).astype(np.float32)
    deg_in = np.maximum(np.bincount(dst, minlength=N_NODES), 1).astype(np.float32)
    node_core = graph_ids // GPC

    # pre-commit each node to half A(0)/B(1): per core, alternate over nodes
    # sorted by out-degree so both the source split and node counts balance
    half = np.zeros(N_NODES, dtype=np.int64)
    for c in range(NCORES):
        n = np.nonzero(node_core == c)[0]
        order = n[np.argsort(-deg_out[n], kind="stable")]
        half[order[1::2]] = 1

    src_half = half[src]
    dA = np.bincount(dst[src_half == 0], minlength=N_NODES)
    dB = np.bincount(dst[src_half == 1], minlength=N_NODES)

    # pack each (core, half) separately
    packs = {}
    Ghalf = 0
    for c in range(NCORES):
        for hh in range(2):
            n = np.nonzero((node_core == c) & (half == hh))[0]
            g, ng = _pack_groups(dA[n], dB[n])
            packs[(c, hh)] = (n, g)
            Ghalf = max(Ghalf, ng)
    Ghalf = -(-Ghalf // 4) * 4            # multiple of 4: NSUP even, pair
                                          # P/2 boundary on a super boundary
    G = 2 * Ghalf
    P = G // 2                            # pairs (128-node tiles)
    NSUP = G // 4
    SH = G * GSZ                          # rows per core shard
    HSH = SH // 2
    RT = NCORES * HSH                     # rows per half-table
    assert RT <= 32767, f"int16 overflow: {RT}"

    # node -> loc (row within core shard)
    loc = np.full(N_NODES, -1, dtype=np.int64)
    for c in range(NCORES):
        for hh in range(2):
            n, g = packs[(c, hh)]
            order = np.lexsort((n, g))
            n_s, g_s = n[order], g[order]
            slot = np.zeros(len(n), dtype=np.int64)
            _, starts = np.unique(g_s, return_index=True)
            for s0, s1 in zip(starts, list(starts[1:]) + [len(n)]):
                slot[s0:s1] = np.arange(s1 - s0)
            loc[n_s] = (hh * Ghalf + g_s) * GSZ + slot

    lochalf = loc % HSH                   # row within own (core, half) shard
    QSH = HSH // 2
    srow = ((lochalf >= QSH).astype(np.int64) * (RT // 2)
            + node_core * QSH + (lochalf % QSH))  # quarter-major half-table row

    # per-core edge layout
    e_core = node_core[dst]
    e_group = loc[dst] // GSZ
    e_dslot = loc[dst] % GSZ
    e_srow = srow[src]

    per_core = []
    for c in range(NCORES):
        m = np.nonzero(e_core == c)[0]
        g, h, sr, dslt = e_group[m], src_half[m], e_srow[m], e_dslot[m]
        order = np.lexsort((sr, h, g))
        g, h, sr, dslt = g[order], h[order], sr[order], dslt[order]
        key = g * 2 + h
        rank = np.arange(len(m)) - np.searchsorted(key, key, side="left")
        k = rank // 128
        p = rank % 128
        assert (k < CHUNKS_PER_HALF).all(), "cap exceeded"
        s = g // 4
        gi = g % 4
        c16 = gi * CHUNKS_PER_HALF + k        # chunk col within (super, half)
        j = c16 * 128 + p                     # slot within (super, half)
        t = s * 2 + h

        idx16 = np.zeros((2 * NSUP, 16, 128), dtype=np.int16)
        idx16[t, j % 16, j // 16] = sr.astype(np.int16)
        idx_all = np.tile(idx16, (1, 8, 1)).reshape(2 * NSUP, 128, 128)
        idx_2d = idx_all.transpose(1, 0, 2).reshape(128, 2 * NSUP * 128).copy()

        dstv = np.full((128, 2 * NSUP * 16), -1.0, dtype=np.float32)
        dstv[j % 128, t * 16 + c16] = dslt.astype(np.float32)

        # per-pair node scalars [128, P]
        nodes_c = np.nonzero(node_core == c)[0]
        lr = loc[nodes_c]
        ni_t = np.ones((128, P), dtype=np.float32)
        no_t = np.ones((128, P), dtype=np.float32)
        gid_t = np.full((128, P), -1, dtype=np.int64)
        pr = lr // 128
        pp = lr % 128
        ni_t[pp, pr] = 1.0 / np.sqrt(deg_in[nodes_c])
        no_t[pp, pr] = 1.0 / np.sqrt(deg_out[nodes_c])
        gid_t[pp, pr] = graph_ids[nodes_c] - c * GPC
        groh = np.zeros((128, P, GPC), dtype=np.float32)
        pg, prr = np.nonzero(gid_t >= 0)
        groh[pg, prr, gid_t[pg, prr]] = 1.0

        rc = (1.0 / np.maximum(
            np.bincount(graph_ids[nodes_c] - c * GPC, minlength=GPC), 1
        )).astype(np.float32).reshape(GPC, 1)

        per_core.append(dict(
            idx=idx_2d, dstv=_to_bf16(dstv), ni=ni_t, no=no_t,
            groh=_to_bf16(groh), rc=rc,
        ))

    # layer-0 half-tables: hn0 = feats * norm_out, bf16, in AllGather layout
    hn0 = feats * (1.0 / np.sqrt(deg_out))[:, None]
    t0 = np.zeros((2, RT, D), dtype=np.float32)
    nodes = np.nonzero(loc >= 0)[0]
    t0[half[nodes], srow[nodes]] = hn0[nodes]
    table0A = _to_bf16(t0[0])
    table0B = _to_bf16(t0[1])

    b_rep = np.broadcast_to(
        np.asarray(b, dtype=np.float32)[None, :, :], (128, N_LAYERS, D)
    ).copy()
    W_t = _to_bf16(np.ascontiguousarray(
        np.asarray(W, dtype=np.float32).transpose(1, 0, 2)))

    meta = dict(G=G, P=P, NSUP=NSUP, SH=SH, HSH=HSH, RT=RT)
    shared = dict(W=W_t, b_rep=b_rep, t0A=table0A, t0B=table0B)
    return per_core, shared, meta


# ------------------------------------------------------------- device build
def build(meta, rep=1, no_coll=False, mode="full", nq=NQ, pieces=PIECES,
          gbufs=4, obufs=8, pbufs=5, sbufs=8, half_mm=False, dbl_oh=False):
    G, P, NSUP, SH, HSH, RT = (meta[k] for k in ("G", "P", "NSUP", "SH", "HSH", "RT"))
    CH = CHUNKS_PER_HALF
    f32 = mybir.dt.float32
    bf16 = mybir.dt.bfloat16

    nc = bacc.Bacc("TRN2", target_bir_lowering=False, debug=False,
                   num_devices=NCORES, dynamic_dma_scratch_size=16384,
                   num_swdge_queues=nq)

    idx_t = nc.dram_tensor("idx", [128, 2 * NSUP * 128], mybir.dt.int16, kind="ExternalInput")
    dstv_t = nc.dram_tensor("dstv", [128, 2 * NSUP * 16], bf16, kind="ExternalInput")
    ni_t = nc.dram_tensor("ni", [128, P], f32, kind="ExternalInput")
    no_t = nc.dram_tensor("no", [128, P], f32, kind="ExternalInput")
    groh_t = nc.dram_tensor("groh", [128, P, GPC], bf16, kind="ExternalInput")
    rc_t = nc.dram_tensor("rc", [GPC, 1], f32, kind="ExternalInput")
    t0A_t = nc.dram_tensor("t0A", [RT, D], bf16, kind="ExternalInput")
    t0B_t = nc.dram_tensor("t0B", [RT, D], bf16, kind="ExternalInput")
    W_t = nc.dram_tensor("W", [128, N_LAYERS, D], bf16, kind="ExternalInput")
    brep_t = nc.dram_tensor("b_rep", [128, N_LAYERS, D], f32, kind="ExternalInput")
    out_t = nc.dram_tensor("out", [GPC, D], f32, kind="ExternalOutput")

    # AllGather outputs for layers 1, 2: [half][RT, D]
    ag = [[nc.dram_tensor(f"ag{l}_{q}", [RT, D], bf16,
                          kind="Internal", addr_space="Shared") for q in range(2)]
          for l in range(1, N_LAYERS)]

    SCOL = (2048 // pieces) // 16          # idx cols per gather piece
    CPP = (CH * 4) // pieces               # chunks per gather piece

    with tile.TileContext(nc) as tc, ExitStack() as ctx:
        dram = ctx.enter_context(tc.tile_pool(name="dram", bufs=1, space="DRAM"))
        stat = ctx.enter_context(tc.tile_pool(name="stat", bufs=1))
        gpool = ctx.enter_context(tc.tile_pool(name="gath", bufs=gbufs))
        opool = ctx.enter_context(tc.tile_pool(name="oh", bufs=obufs))
        spool = ctx.enter_context(tc.tile_pool(name="sb", bufs=sbufs))
        ppool = ctx.enter_context(tc.tile_pool(name="agg_ps", bufs=pbufs, space="PSUM"))
        hpool = ctx.enter_context(tc.tile_pool(name="h_ps", bufs=2, space="PSUM"))
        plpool = ctx.enter_context(tc.tile_pool(name="pool_ps", bufs=1, space="PSUM"))

        hn_shard = dram.tile([SH, D], bf16)

        # ---- statics
        idx_sb = stat.tile([128, 2 * NSUP * 128], mybir.dt.int16)
        nc.sync.dma_start(idx_sb[:], idx_t.ap())
        dstv_sb = stat.tile([128, 2 * NSUP * 16], bf16)
        nc.sync.dma_start(dstv_sb[:], dstv_t.ap())
        W_sb = stat.tile([128, N_LAYERS, D], bf16)
        nc.sync.dma_start(W_sb[:], W_t.ap())
        brep_sb = stat.tile([128, N_LAYERS, D], f32)
        nc.sync.dma_start(brep_sb[:], brep_t.ap())
        groh_sb = stat.tile([128, P, GPC], bf16)
        nc.sync.dma_start(groh_sb[:], groh_t.ap())
        ni_sb = stat.tile([128, P], f32)
        nc.sync.dma_start(ni_sb[:], ni_t.ap())
        no_sb = stat.tile([128, P], f32)
        nc.sync.dma_start(no_sb[:], no_t.ap())
        rc_sb = stat.tile([GPC, 1], f32)
        nc.sync.dma_start(rc_sb[:], rc_t.ap())

        iota16 = stat.tile([128, GSZ], mybir.dt.int16)
        nc.gpsimd.iota(iota16[:], pattern=[[1, GSZ]], base=0, channel_multiplier=0)
        iota_f = stat.tile([128, GSZ], bf16)
        nc.vector.tensor_copy(iota_f[:], iota16[:])

        aggA_sb = stat.tile([128, P, 128], bf16)
        aggB_sb = stat.tile([128, P, 128], bf16)
        if mode == "compute_only":
            gstatA = stat.tile([128, 4 * CH, D], bf16)
            nc.vector.memset(gstatA[:], 0.25)
            gstatB = stat.tile([128, 4 * CH, D], bf16)
            nc.vector.memset(gstatB[:], 0.25)

        qctr = [0]

        def gather_half_fn(gt, src_ap, s, hh):
            base_col = (2 * s + hh) * 128
            for piece in range(pieces):
                nc.gpsimd.dma_gather(
                    out_ap=gt[:, piece * CPP:(piece + 1) * CPP, :],
                    in_ap=src_ap,
                    idxs_ap=idx_sb[:, base_col + piece * SCOL:
                                   base_col + (piece + 1) * SCOL],
                    num_idxs=2048 // pieces, num_idxs_reg=2048 // pieces,
                    elem_size=D, single_packet=False,
                    queue_num=qctr[0] % nq,
                )
                qctr[0] += 1

        def build_oh(s, hh, tag):
            oh = opool.tile([128, 4 * CH, GSZ], bf16, tag=tag)
            t16 = (2 * s + hh) * 16
            for _r in range(2 if dbl_oh else 1):
                nc.vector.tensor_tensor(
                    out=oh[:],
                    in0=iota_f[:].unsqueeze(1).broadcast_to([128, 4 * CH, GSZ]),
                    in1=dstv_sb[:, t16:t16 + 16]
                        .unsqueeze(2).broadcast_to([128, 4 * CH, GSZ]),
                    op=mybir.AluOpType.is_equal,
                )
            return oh

        def seg_matmuls(gt, oh, s, agg_dst):
            """8 matmuls per pair accumulating [f, dslot] into agg PSUM,
            then evict to agg_dst[:, pr, :] (bf16)."""
            for pi in range(2):
                pr = s * 2 + pi
                agg = ppool.tile([128, 128], f32, tag="agg")
                for gj in range(2):
                    gi = pi * 2 + gj
                    off = gj * GSZ
                    ks = range(0, CH, 2) if half_mm else range(CH)
                    last = list(ks)[-1]
                    for k in ks:
                        nc.tensor.matmul(
                            agg[:, off:off + GSZ],
                            gt[:, gi * CH + k, :],
                            oh[:, gi * CH + k, :],
                            start=(k == 0), stop=(k == last and gj == 1),
                            skip_group_check=True,
                        )
                nc.scalar.copy(agg_dst[:, pr, :], agg[:])

        gather_half = gather_half_fn if mode != "compute_only" else (lambda *a: None)
        if mode == "gather_only":
            acc = stat.tile([128, 1], f32)
            nc.vector.memset(acc[:], 0.0)
            dump = stat.tile([GPC, D], f32)
            nc.vector.memset(dump[:], 0.0)
            for _ in range(rep):
                for l in range(N_LAYERS):
                    for hh in range(2):
                        src_ap = (t0A_t.ap() if hh == 0 else t0B_t.ap())
                        for s in range(NSUP):
                            gt = gpool.tile([128, 4 * CH, D], bf16, tag="g")
                            gather_half_fn(gt, src_ap, s, hh)
                            for piece in range(pieces):
                                nc.vector.tensor_tensor(
                                    out=acc[:], in0=acc[:],
                                    in1=gt[:, piece * CPP, 0:1],
                                    op=mybir.AluOpType.add)
            nc.sync.dma_start(out_t.ap(), dump[:])
            rep = 0
        for _ in range(rep):
            pool_ps = plpool.tile([GPC, D], f32)

            for l in range(N_LAYERS):
                if l == 0:
                    srcA, srcB = t0A_t.ap(), t0B_t.ap()
                else:
                    srcA, srcB = ag[l - 1][0].ap(), ag[l - 1][1].ap()

                # phase A: all supers' A-half work
                for s in range(NSUP):
                    if mode == "compute_only":
                        gA = gstatA
                    else:
                        gA = gpool.tile([128, 4 * CH, D], bf16, tag="gA")
                        gather_half(gA, srcA, s, 0)
                    ohA = build_oh(s, 0, "ohA")
                    seg_matmuls(gA, ohA, s, aggA_sb)

                # phase B + dense + epilogue
                for s in range(NSUP):
                    if mode == "compute_only":
                        gB = gstatB
                    else:
                        gB = gpool.tile([128, 4 * CH, D], bf16, tag="gB")
                        gather_half(gB, srcB, s, 1)
                    ohB = build_oh(s, 1, "ohB")
                    seg_matmuls(gB, ohB, s, aggB_sb)
                    for pi in range(2):
                        pr = s * 2 + pi
                        hps = hpool.tile([128, D], f32, tag="hps")
                        nc.tensor.matmul(hps[:], aggA_sb[:, pr, :], W_sb[:, l, :],
                                         start=True, stop=False,
                                         skip_group_check=True)
                        nc.tensor.matmul(hps[:], aggB_sb[:, pr, :], W_sb[:, l, :],
                                         start=False, stop=True,
                                         skip_group_check=True)
                        t_sb = spool.tile([128, D], f32, tag="tsb")
                        nc.vector.scalar_tensor_tensor(
                            out=t_sb[:], in0=hps[:], scalar=ni_sb[:, pr:pr + 1],
                            in1=brep_sb[:, l, :],
                            op0=mybir.AluOpType.mult, op1=mybir.AluOpType.add,
                        )
                        if l < N_LAYERS - 1:
                            hn = spool.tile([128, D], bf16, tag="hn")
                            nc.vector.tensor_scalar(
                                out=hn[:], in0=t_sb[:],
                                scalar1=0.0, scalar2=no_sb[:, pr:pr + 1],
                                op0=mybir.AluOpType.max, op1=mybir.AluOpType.mult,
                            )
                            nc.sync.dma_start(
                                hn_shard[pr * 128:(pr + 1) * 128, :], hn[:])
                        else:
                            h_sb = spool.tile([128, D], bf16, tag="hsb")
                            nc.vector.tensor_scalar_max(h_sb[:], t_sb[:], 0.0)
                            nc.tensor.matmul(
                                pool_ps[:], groh_sb[:, pr, :], h_sb[:],
                                start=(pr == 0), stop=(pr == P - 1),
                                skip_group_check=True,
                            )
                    # fire next layer's AllGathers quarter-by-quarter as
                    # soon as each input slice is fully written
                    if l < N_LAYERS - 1 and not no_coll and mode == "full":
                        QSH = HSH // 2
                        for qi in range(4):           # quarter qi = rows
                            # [qi*QSH, (qi+1)*QSH) -> ag[half][q2 slice]
                            need_pairs = ((qi + 1) * QSH + 127) // 128
                            fire_s = (need_pairs + 1) // 2 - 1
                            if s == fire_s:
                                hhq, q2 = qi // 2, qi % 2
                                nc.gpsimd.collective_compute(
                                    "AllGather", mybir.AluOpType.bypass,
                                    replica_groups=[list(range(NCORES))],
                                    ins=[hn_shard[qi * QSH:(qi + 1) * QSH, :].opt()],
                                    outs=[ag[l][hhq].ap()
                                          [q2 * (RT // 2):(q2 + 1) * (RT // 2), :].opt()],
                                )

            pool_sb = spool.tile([GPC, D], f32, tag="poolsb")
            nc.vector.tensor_scalar_mul(pool_sb[:], pool_ps[:], rc_sb[:])
            nc.sync.dma_start(out_t.ap(), pool_sb[:])

    nc.compile()
    return nc


def make_in_maps(per_core, shared):
    in_maps = []
    for c in range(NCORES):
        pc = per_core[c]
        in_maps.append({
            "idx": pc["idx"], "dstv": pc["dstv"], "ni": pc["ni"],
            "no": pc["no"], "groh": pc["groh"], "rc": pc["rc"],
            "t0A": shared["t0A"], "t0B": shared["t0B"],
            "W": shared["W"], "b_rep": shared["b_rep"],
        })
    return in_maps


def kernel(**inputs) -> np.ndarray:
    per_core, shared, meta = preprocess(**inputs)
    nc = build(meta, rep=1)
    in_maps = make_in_maps(per_core, shared)
    res = run_bass_kernel_spmd(nc, in_maps, core_ids=list(range(NCORES)))
    return np.concatenate([res.results[c]["out"] for c in range(NCORES)], axis=0)


# revision 15
# speedup vs baseline: 6.0253x; 1.3046x over previous
"""BRPConvEmbedding (3-layer GraphConv + AvgPool readout) on 8 Trainium2 cores.

Sharding: graphs split contiguously across cores (32 graphs/core); each core
owns its graphs' nodes, so pooling is core-local and the output is a concat.
Within a core, nodes are pre-committed to two halves (A/B) and each half is
packed into dst-groups of <=64 nodes whose in-degree per source-half is capped
at 512 (4 chunks of 128 edge slots), giving a uniform SPMD layout.

Per layer the full node-feature table lives in two bf16 halves (all cores'
A-rows / B-rows), built by one AllGather each; layer 0's halves are
host-precomputed (feats * norm_out) and passed in, so layer 0 needs no
collective. SpMM: per-edge rows are fetched with SWDGE dma_gather (int16
indices, 4 parallel queues), the per-edge one-hot is built on the VectorE, and
the segment-sum runs on the TensorE via gathered-chunk x one-hot matmuls in
bf16 with fp32 PSUM accumulation. The A-half runs for all supers first (so the
B-half AllGather overlaps compute); each pair's A/B partial aggregates are
evicted to SBUF and combined by two accumulating W-matmuls, then the epilogue
(x norm_in, +b, relu, x norm_out) writes bf16 rows for the next AllGather.
"""
import numpy as np
from contextlib import ExitStack

import concourse.bacc as bacc
import concourse.mybir as mybir
from concourse import tile
from concourse.bass_utils import run_bass_kernel_spmd

N_NODES = 50000
N_EDGES = 800000
D = 128
N_LAYERS = 3
N_GRAPHS = 256
NCORES = 8
GSZ = 64                  # dst nodes per group
CHUNKS_PER_HALF = 4       # 4*128 = 512 edge-slot cap per (group, src-half)
CAP = CHUNKS_PER_HALF * 128
GPC = N_GRAPHS // NCORES  # graphs per core
NQ = 4                    # SWDGE queues
PIECES = 4                # gather instructions per (super, half)


# ----------------------------------------------------------------- host prep
def _pack_groups(dA, dB):
    """Greedy bin-packing: nodes (rows of dA/dB) into groups of <= GSZ nodes
    with sum(dA) <= CAP and sum(dB) <= CAP. Returns group ids."""
    n = len(dA)
    order = np.argsort(-np.maximum(dA, dB), kind="stable")
    gids = np.full(n, -1, dtype=np.int64)
    usedA, usedB, usedN = [], [], []
    for i in order:
        a, b = dA[i], dB[i]
        best, best_fit = -1, -1.0
        for g in range(len(usedA)):
            if usedN[g] < GSZ and usedA[g] + a <= CAP and usedB[g] + b <= CAP:
                fit = max((usedA[g] + a) / CAP, (usedB[g] + b) / CAP)
                if fit > best_fit:
                    best, best_fit = g, fit
        if best < 0:
            usedA.append(0), usedB.append(0), usedN.append(0)
            best = len(usedA) - 1
        gids[i] = best
        usedA[best] += a
        usedB[best] += b
        usedN[best] += 1
    return gids, len(usedA)


def _to_bf16(x):
    import jax.numpy as jnp
    return np.asarray(jnp.asarray(np.asarray(x, np.float32), dtype=jnp.bfloat16))


def preprocess(feats, W, b, src, dst, graph_ids):
    src = np.asarray(src).astype(np.int64)
    dst = np.asarray(dst).astype(np.int64)
    graph_ids = np.asarray(graph_ids).astype(np.int64)
    feats = np.asarray(feats, dtype=np.float32)

    deg_out = np.maximum(np.bincount(src, minlength=N_NODES), 1).astype(np.float32)
    deg_in = np.maximum(np.bincount(dst, minlength=N_NODES), 1).astype(np.float32)
    node_core = graph_ids // GPC

    # pre-commit each node to half A(0)/B(1): per core, alternate over nodes
    # sorted by out-degree so both the source split and node counts balance
    half = np.zeros(N_NODES, dtype=np.int64)
    for c in range(NCORES):
        n = np.nonzero(node_core == c)[0]
        order = n[np.argsort(-deg_out[n], kind="stable")]
        half[order[1::2]] = 1

    src_half = half[src]
    dA = np.bincount(dst[src_half == 0], minlength=N_NODES)
    dB = np.bincount(dst[src_half == 1], minlength=N_NODES)

    # pack each (core, half) separately
    packs = {}
    Ghalf = 0
    for c in range(NCORES):
        for hh in range(2):
            n = np.nonzero((node_core == c) & (half == hh))[0]
            g, ng = _pack_groups(dA[n], dB[n])
            packs[(c, hh)] = (n, g)
            Ghalf = max(Ghalf, ng)
    Ghalf = -(-Ghalf // 4) * 4            # multiple of 4: NSUP even, pair
                                          # P/2 boundary on a super boundary
    G = 2 * Ghalf
    P = G // 2                            # pairs (128-node tiles)
    NSUP = G // 4
    SH = G * GSZ                          # rows per core shard
    HSH = SH // 2
    RT = NCORES * HSH                     # rows per half-table
    assert RT <= 32767, f"int16 overflow: {RT}"

    # node -> loc (row within core shard)
    loc = np.full(N_NODES, -1, dtype=np.int64)
    for c in range(NCORES):
        for hh in range(2):
            n, g = packs[(c, hh)]
            order = np.lexsort((n, g))
            n_s, g_s = n[order], g[order]
            slot = np.zeros(len(n), dtype=np.int64)
            _, starts = np.unique(g_s, return_index=True)
            for s0, s1 in zip(starts, list(starts[1:]) + [len(n)]):
                slot[s0:s1] = np.arange(s1 - s0)
            loc[n_s] = (hh * Ghalf + g_s) * GSZ + slot

    lochalf = loc % HSH                   # row within own (core, half) shard
    QSH = HSH // 2
    srow = ((lochalf >= QSH).astype(np.int64) * (RT // 2)
            + node_core * QSH + (lochalf % QSH))  # quarter-major half-table row

    # per-core edge layout
    e_core = node_core[dst]
    e_group = loc[dst] // GSZ
    e_dslot = loc[dst] % GSZ
    e_srow = srow[src]

    per_core = []
    for c in range(NCORES):
        m = np.nonzero(e_core == c)[0]
        g, h, sr, dslt = e_group[m], src_half[m], e_srow[m], e_dslot[m]
        order = np.lexsort((sr, h, g))
        g, h, sr, dslt = g[order], h[order], sr[order], dslt[order]
        key = g * 2 + h
        rank = np.arange(len(m)) - np.searchsorted(key, key, side="left")
        k = rank // 128
        p = rank % 128
        assert (k < CHUNKS_PER_HALF).all(), "cap exceeded"
        s = g // 4
        gi = g % 4
        c16 = gi * CHUNKS_PER_HALF + k        # chunk col within (super, half)
        j = c16 * 128 + p                     # slot within (super, half)
        t = s * 2 + h

        idx16 = np.zeros((2 * NSUP, 16, 128), dtype=np.int16)
        idx16[t, j % 16, j // 16] = sr.astype(np.int16)
        idx_all = np.tile(idx16, (1, 8, 1)).reshape(2 * NSUP, 128, 128)
        idx_2d = idx_all.transpose(1, 0, 2).reshape(128, 2 * NSUP * 128).copy()

        dstv = np.full((128, 2 * NSUP * 16), -1.0, dtype=np.float32)
        dstv[j % 128, t * 16 + c16] = dslt.astype(np.float32)

        # per-pair node scalars [128, P]
        nodes_c = np.nonzero(node_core == c)[0]
        lr = loc[nodes_c]
        ni_t = np.ones((128, P), dtype=np.float32)
        no_t = np.ones((128, P), dtype=np.float32)
        gid_t = np.full((128, P), -1, dtype=np.int64)
        pr = lr // 128
        pp = lr % 128
        ni_t[pp, pr] = 1.0 / np.sqrt(deg_in[nodes_c])
        no_t[pp, pr] = 1.0 / np.sqrt(deg_out[nodes_c])
        gid_t[pp, pr] = graph_ids[nodes_c] - c * GPC
        groh = np.zeros((128, P, GPC), dtype=np.float32)
        pg, prr = np.nonzero(gid_t >= 0)
        groh[pg, prr, gid_t[pg, prr]] = 1.0

        rc = (1.0 / np.maximum(
            np.bincount(graph_ids[nodes_c] - c * GPC, minlength=GPC), 1
        )).astype(np.float32).reshape(GPC, 1)

        per_core.append(dict(
            idx=idx_2d, dstv=_to_bf16(dstv), ni=ni_t, no=no_t,
            groh=_to_bf16(groh), rc=rc,
        ))

    # layer-0 half-tables: hn0 = feats * norm_out, bf16, in AllGather layout
    hn0 = feats * (1.0 / np.sqrt(deg_out))[:, None]
    t0 = np.zeros((2, RT, D), dtype=np.float32)
    nodes = np.nonzero(loc >= 0)[0]
    t0[half[nodes], srow[nodes]] = hn0[nodes]
    table0A = _to_bf16(t0[0])
    table0B = _to_bf16(t0[1])

    b_rep = np.broadcast_to(
        np.asarray(b, dtype=np.float32)[None, :, :], (128, N_LAYERS, D)
    ).copy()
    W_t = _to_bf16(np.ascontiguousarray(
        np.asarray(W, dtype=np.float32).transpose(1, 0, 2)))

    meta = dict(G=G, P=P, NSUP=NSUP, SH=SH, HSH=HSH, RT=RT)
    shared = dict(W=W_t, b_rep=b_rep, t0A=table0A, t0B=table0B)
    return per_core, shared, meta


# ------------------------------------------------------------- device build
def build(meta, rep=1, no_coll=False, mode="full", nq=NQ, pieces=PIECES,
          gbufs=4, obufs=12, pbufs=5, sbufs=8, half_mm=False, dbl_oh=False,
          spkt=False):
    G, P, NSUP, SH, HSH, RT = (meta[k] for k in ("G", "P", "NSUP", "SH", "HSH", "RT"))
    CH = CHUNKS_PER_HALF
    f32 = mybir.dt.float32
    bf16 = mybir.dt.bfloat16

    nc = bacc.Bacc("TRN2", target_bir_lowering=False, debug=False,
                   num_devices=NCORES, dynamic_dma_scratch_size=16384,
                   num_swdge_queues=nq)

    idx_t = nc.dram_tensor("idx", [128, 2 * NSUP * 128], mybir.dt.int16, kind="ExternalInput")
    dstv_t = nc.dram_tensor("dstv", [128, 2 * NSUP * 16], bf16, kind="ExternalInput")
    ni_t = nc.dram_tensor("ni", [128, P], f32, kind="ExternalInput")
    no_t = nc.dram_tensor("no", [128, P], f32, kind="ExternalInput")
    groh_t = nc.dram_tensor("groh", [128, P, GPC], bf16, kind="ExternalInput")
    rc_t = nc.dram_tensor("rc", [GPC, 1], f32, kind="ExternalInput")
    t0A_t = nc.dram_tensor("t0A", [RT, D], bf16, kind="ExternalInput")
    t0B_t = nc.dram_tensor("t0B", [RT, D], bf16, kind="ExternalInput")
    W_t = nc.dram_tensor("W", [128, N_LAYERS, D], bf16, kind="ExternalInput")
    brep_t = nc.dram_tensor("b_rep", [128, N_LAYERS, D], f32, kind="ExternalInput")
    out_t = nc.dram_tensor("out", [GPC, D], f32, kind="ExternalOutput")

    # AllGather outputs for layers 1, 2: [half][RT, D]
    ag = [[nc.dram_tensor(f"ag{l}_{q}", [RT, D], bf16,
                          kind="Internal", addr_space="Shared") for q in range(2)]
          for l in range(1, N_LAYERS)]

    SCOL = (2048 // pieces) // 16          # idx cols per gather piece
    CPP = (CH * 4) // pieces               # chunks per gather piece

    with tile.TileContext(nc) as tc, ExitStack() as ctx:
        dram = ctx.enter_context(tc.tile_pool(name="dram", bufs=1, space="DRAM"))
        stat = ctx.enter_context(tc.tile_pool(name="stat", bufs=1))
        gpool = ctx.enter_context(tc.tile_pool(name="gath", bufs=gbufs))
        opool = ctx.enter_context(tc.tile_pool(name="oh", bufs=obufs))
        spool = ctx.enter_context(tc.tile_pool(name="sb", bufs=sbufs))
        ppool = ctx.enter_context(tc.tile_pool(name="agg_ps", bufs=pbufs, space="PSUM"))
        hpool = ctx.enter_context(tc.tile_pool(name="h_ps", bufs=2, space="PSUM"))
        plpool = ctx.enter_context(tc.tile_pool(name="pool_ps", bufs=1, space="PSUM"))

        hn_shard = dram.tile([SH, D], bf16)

        # ---- statics
        idx_sb = stat.tile([128, 2 * NSUP * 128], mybir.dt.int16)
        nc.sync.dma_start(idx_sb[:], idx_t.ap())
        dstv_sb = stat.tile([128, 2 * NSUP * 16], bf16)
        nc.sync.dma_start(dstv_sb[:], dstv_t.ap())
        W_sb = stat.tile([128, N_LAYERS, D], bf16)
        nc.sync.dma_start(W_sb[:], W_t.ap())
        brep_sb = stat.tile([128, N_LAYERS, D], f32)
        nc.sync.dma_start(brep_sb[:], brep_t.ap())
        groh_sb = stat.tile([128, P, GPC], bf16)
        nc.sync.dma_start(groh_sb[:], groh_t.ap())
        ni_sb = stat.tile([128, P], f32)
        nc.sync.dma_start(ni_sb[:], ni_t.ap())
        no_sb = stat.tile([128, P], f32)
        nc.sync.dma_start(no_sb[:], no_t.ap())
        rc_sb = stat.tile([GPC, 1], f32)
        nc.sync.dma_start(rc_sb[:], rc_t.ap())

        iota16 = stat.tile([128, GSZ], mybir.dt.int16)
        nc.gpsimd.iota(iota16[:], pattern=[[1, GSZ]], base=0, channel_multiplier=0)
        iota_f = stat.tile([128, GSZ], bf16)
        nc.vector.tensor_copy(iota_f[:], iota16[:])

        aggA_sb = stat.tile([128, P, 128], bf16)
        aggB_sb = stat.tile([128, P, 128], bf16)
        if mode == "compute_only":
            gstatA = stat.tile([128, 4 * CH, D], bf16)
            nc.vector.memset(gstatA[:], 0.25)
            gstatB = stat.tile([128, 4 * CH, D], bf16)
            nc.vector.memset(gstatB[:], 0.25)

        qctr = [0]

        def gather_half_fn(gt, src_ap, s, hh):
            base_col = (2 * s + hh) * 128
            for piece in range(pieces):
                nc.gpsimd.dma_gather(
                    out_ap=gt[:, piece * CPP:(piece + 1) * CPP, :],
                    in_ap=src_ap,
                    idxs_ap=idx_sb[:, base_col + piece * SCOL:
                                   base_col + (piece + 1) * SCOL],
                    num_idxs=2048 // pieces, num_idxs_reg=2048 // pieces,
                    elem_size=D, single_packet=spkt,
                    queue_num=qctr[0] % nq,
                )
                qctr[0] += 1

        def build_oh(s, hh, tag):
            oh = opool.tile([128, 4 * CH, GSZ], bf16, tag=tag)
            t16 = (2 * s + hh) * 16
            for _r in range(2 if dbl_oh else 1):
                nc.vector.tensor_tensor(
                    out=oh[:],
                    in0=iota_f[:].unsqueeze(1).broadcast_to([128, 4 * CH, GSZ]),
                    in1=dstv_sb[:, t16:t16 + 16]
                        .unsqueeze(2).broadcast_to([128, 4 * CH, GSZ]),
                    op=mybir.AluOpType.is_equal,
                )
            return oh

        def seg_matmuls(gt, oh, s, agg_dst):
            """8 matmuls per pair accumulating [f, dslot] into agg PSUM,
            then evict to agg_dst[:, pr, :] (bf16)."""
            for pi in range(2):
                pr = s * 2 + pi
                agg = ppool.tile([128, 128], f32, tag="agg")
                for gj in range(2):
                    gi = pi * 2 + gj
                    off = gj * GSZ
                    ks = range(0, CH, 2) if half_mm else range(CH)
                    last = list(ks)[-1]
                    for k in ks:
                        nc.tensor.matmul(
                            agg[:, off:off + GSZ],
                            gt[:, gi * CH + k, :],
                            oh[:, gi * CH + k, :],
                            start=(k == 0), stop=(k == last and gj == 1),
                            skip_group_check=True,
                        )
                nc.scalar.copy(agg_dst[:, pr, :], agg[:])

        gather_half = gather_half_fn if mode != "compute_only" else (lambda *a: None)
        if mode == "gather_only":
            acc = stat.tile([128, 1], f32)
            nc.vector.memset(acc[:], 0.0)
            dump = stat.tile([GPC, D], f32)
            nc.vector.memset(dump[:], 0.0)
            for _ in range(rep):
                for l in range(N_LAYERS):
                    for hh in range(2):
                        src_ap = (t0A_t.ap() if hh == 0 else t0B_t.ap())
                        for s in range(NSUP):
                            gt = gpool.tile([128, 4 * CH, D], bf16, tag="g")
                            gather_half_fn(gt, src_ap, s, hh)
                            for piece in range(pieces):
                                nc.vector.tensor_tensor(
                                    out=acc[:], in0=acc[:],
                                    in1=gt[:, piece * CPP, 0:1],
                                    op=mybir.AluOpType.add)
            nc.sync.dma_start(out_t.ap(), dump[:])
            rep = 0
        for _ in range(rep):
            pool_ps = plpool.tile([GPC, D], f32)

            for l in range(N_LAYERS):
                if l == 0:
                    srcA, srcB = t0A_t.ap(), t0B_t.ap()
                else:
                    srcA, srcB = ag[l - 1][0].ap(), ag[l - 1][1].ap()

                # layer 0 has both tables at launch: single-pass supers
                # (one PSUM group of 32 matmuls, one evict, one dense)
                single = (l == 0 and mode == "full")
                if not single:
                    # phase A: all supers' A-half work
                    for s in range(NSUP):
                        if mode == "compute_only":
                            gA = gstatA
                        else:
                            gA = gpool.tile([128, 4 * CH, D], bf16, tag="gA")
                            gather_half(gA, srcA, s, 0)
                        ohA = build_oh(s, 0, "ohA")
                        seg_matmuls(gA, ohA, s, aggA_sb)

                # phase B + dense + epilogue
                for s in range(NSUP):
                    if mode == "compute_only":
                        gB = gstatB
                    else:
                        gB = gpool.tile([128, 4 * CH, D], bf16, tag="gB")
                        gather_half(gB, srcB, s, 1)
                    ohB = build_oh(s, 1, "ohB")
                    if single:
                        gA = gpool.tile([128, 4 * CH, D], bf16, tag="gA")
                        gather_half(gA, srcA, s, 0)
                        ohA = build_oh(s, 0, "ohA")
                        for pi in range(2):
                            pr = s * 2 + pi
                            agg = ppool.tile([128, 128], f32, tag="agg")
                            for hi, (gt, oh) in enumerate(((gA, ohA), (gB, ohB))):
                                for gj in range(2):
                                    gi = pi * 2 + gj
                                    off = gj * GSZ
                                    for k in range(CH):
                                        nc.tensor.matmul(
                                            agg[:, off:off + GSZ],
                                            gt[:, gi * CH + k, :],
                                            oh[:, gi * CH + k, :],
                                            start=(hi == 0 and gj == 0 and k == 0),
                                            stop=(hi == 1 and gj == 1 and k == CH - 1),
                                            skip_group_check=True,
                                        )
                            nc.scalar.copy(aggB_sb[:, pr, :], agg[:])
                    else:
                        seg_matmuls(gB, ohB, s, aggB_sb)
                    for pi in range(2):
                        pr = s * 2 + pi
                        hps = hpool.tile([128, D], f32, tag="hps")
                        if single:
                            nc.tensor.matmul(hps[:], aggB_sb[:, pr, :],
                                             W_sb[:, l, :], start=True, stop=True,
                                             skip_group_check=True)
                        else:
                            nc.tensor.matmul(hps[:], aggA_sb[:, pr, :],
                                             W_sb[:, l, :], start=True, stop=False,
                                             skip_group_check=True)
                            nc.tensor.matmul(hps[:], aggB_sb[:, pr, :],
                                             W_sb[:, l, :], start=False, stop=True,
                                             skip_group_check=True)
                        t_sb = spool.tile([128, D], f32, tag="tsb")
                        nc.vector.scalar_tensor_tensor(
                            out=t_sb[:], in0=hps[:], scalar=ni_sb[:, pr:pr + 1],
                            in1=brep_sb[:, l, :],
                            op0=mybir.AluOpType.mult, op1=mybir.AluOpType.add,
                        )
                        if l < N_LAYERS - 1:
                            hn = spool.tile([128, D], bf16, tag="hn")
                            nc.vector.tensor_scalar(
                                out=hn[:], in0=t_sb[:],
                                scalar1=0.0, scalar2=no_sb[:, pr:pr + 1],
                                op0=mybir.AluOpType.max, op1=mybir.AluOpType.mult,
                            )
                            nc.sync.dma_start(
                                hn_shard[pr * 128:(pr + 1) * 128, :], hn[:])
                        else:
                            h_sb = spool.tile([128, D], bf16, tag="hsb")
                            nc.vector.tensor_scalar_max(h_sb[:], t_sb[:], 0.0)
                            nc.tensor.matmul(
                                pool_ps[:], groh_sb[:, pr, :], h_sb[:],
                                start=(pr == 0), stop=(pr == P - 1),
                                skip_group_check=True,
                            )
                    # fire next layer's AllGathers quarter-by-quarter as
                    # soon as each input slice is fully written
                    if l < N_LAYERS - 1 and not no_coll and mode == "full":
                        QSH = HSH // 2
                        for qi in range(4):           # quarter qi = rows
                            # [qi*QSH, (qi+1)*QSH) -> ag[half][q2 slice]
                            need_pairs = ((qi + 1) * QSH + 127) // 128
                            fire_s = (need_pairs + 1) // 2 - 1
                            if s == fire_s:
                                hhq, q2 = qi // 2, qi % 2
                                nc.gpsimd.collective_compute(
                                    "AllGather", mybir.AluOpType.bypass,
                                    replica_groups=[list(range(NCORES))],
                                    ins=[hn_shard[qi * QSH:(qi + 1) * QSH, :].opt()],
                                    outs=[ag[l][hhq].ap()
                                          [q2 * (RT // 2):(q2 + 1) * (RT // 2), :].opt()],
                                )

            pool_sb = spool.tile([GPC, D], f32, tag="poolsb")
            nc.vector.tensor_scalar_mul(pool_sb[:], pool_ps[:], rc_sb[:])
            nc.sync.dma_start(out_t.ap(), pool_sb[:])

    nc.compile()
    return nc


def make_in_maps(per_core, shared):
    in_maps = []
    for c in range(NCORES):
        pc = per_core[c]
        in_maps.append({
            "idx": pc["idx"], "dstv": pc["dstv"], "ni": pc["ni"],
            "no": pc["no"], "groh": pc["groh"], "rc": pc["rc"],
            "t0A": shared["t0A"], "t0B": shared["t0B"],
            "W": shared["W"], "b_rep": shared["b_rep"],
        })
    return in_maps


def kernel(**inputs) -> np.ndarray:
    per_core, shared, meta = preprocess(**inputs)
    nc = build(meta, rep=1)
    in_maps = make_in_maps(per_core, shared)
    res = run_bass_kernel_spmd(nc, in_maps, core_ids=list(range(NCORES)))
    return np.concatenate([res.results[c]["out"] for c in range(NCORES)], axis=0)


# revision 17
# speedup vs baseline: 6.4319x; 1.0675x over previous
"""BRPConvEmbedding (3-layer GraphConv + AvgPool readout) on 8 Trainium2 cores.

Sharding: graphs split contiguously across cores (32 graphs/core); each core
owns its graphs' nodes, so pooling is core-local and the output is a concat.
Within a core, nodes are pre-committed to two halves (A/B) and each half is
packed into dst-groups of <=64 nodes whose in-degree per source-half is capped
at 512 (4 chunks of 128 edge slots), giving a uniform SPMD layout.

Per layer the full node-feature table lives in two bf16 halves (all cores'
A-rows / B-rows), built by one AllGather each; layer 0's halves are
host-precomputed (feats * norm_out) and passed in, so layer 0 needs no
collective. SpMM: per-edge rows are fetched with SWDGE dma_gather (int16
indices, 4 parallel queues), the per-edge one-hot is built on the VectorE, and
the segment-sum runs on the TensorE via gathered-chunk x one-hot matmuls in
bf16 with fp32 PSUM accumulation. The A-half runs for all supers first (so the
B-half AllGather overlaps compute); each pair's A/B partial aggregates are
evicted to SBUF and combined by two accumulating W-matmuls, then the epilogue
(x norm_in, +b, relu, x norm_out) writes bf16 rows for the next AllGather.
"""
import numpy as np
from contextlib import ExitStack

import concourse.bacc as bacc
import concourse.mybir as mybir
from concourse import tile
from concourse.bass_utils import run_bass_kernel_spmd

N_NODES = 50000
N_EDGES = 800000
D = 128
N_LAYERS = 3
N_GRAPHS = 256
NCORES = 8
GSZ = 64                  # dst nodes per group
CHUNKS_PER_HALF = 4       # 4*128 = 512 edge-slot cap per (group, src-half)
CAP = CHUNKS_PER_HALF * 128
GPC = N_GRAPHS // NCORES  # graphs per core
NQ = 4                    # SWDGE queues
PIECES = 4                # gather instructions per (super, half)


# ----------------------------------------------------------------- host prep
def _pack_groups(dA, dB):
    """Greedy bin-packing: nodes (rows of dA/dB) into groups of <= GSZ nodes
    with sum(dA) <= CAP and sum(dB) <= CAP. Returns group ids."""
    n = len(dA)
    order = np.argsort(-np.maximum(dA, dB), kind="stable")
    gids = np.full(n, -1, dtype=np.int64)
    usedA, usedB, usedN = [], [], []
    for i in order:
        a, b = dA[i], dB[i]
        best, best_fit = -1, -1.0
        for g in range(len(usedA)):
            if usedN[g] < GSZ and usedA[g] + a <= CAP and usedB[g] + b <= CAP:
                fit = max((usedA[g] + a) / CAP, (usedB[g] + b) / CAP)
                if fit > best_fit:
                    best, best_fit = g, fit
        if best < 0:
            usedA.append(0), usedB.append(0), usedN.append(0)
            best = len(usedA) - 1
        gids[i] = best
        usedA[best] += a
        usedB[best] += b
        usedN[best] += 1
    return gids, len(usedA)


def _to_bf16(x):
    import jax.numpy as jnp
    return np.asarray(jnp.asarray(np.asarray(x, np.float32), dtype=jnp.bfloat16))


def preprocess(feats, W, b, src, dst, graph_ids):
    src = np.asarray(src).astype(np.int64)
    dst = np.asarray(dst).astype(np.int64)
    graph_ids = np.asarray(graph_ids).astype(np.int64)
    feats = np.asarray(feats, dtype=np.float32)

    deg_out = np.maximum(np.bincount(src, minlength=N_NODES), 1).astype(np.float32)
    deg_in = np.maximum(np.bincount(dst, minlength=N_NODES), 1).astype(np.float32)
    node_core = graph_ids // GPC

    # pre-commit each node to half A(0)/B(1): per core, alternate over nodes
    # sorted by out-degree so both the source split and node counts balance
    half = np.zeros(N_NODES, dtype=np.int64)
    for c in range(NCORES):
        n = np.nonzero(node_core == c)[0]
        order = n[np.argsort(-deg_out[n], kind="stable")]
        half[order[1::2]] = 1

    src_half = half[src]
    dA = np.bincount(dst[src_half == 0], minlength=N_NODES)
    dB = np.bincount(dst[src_half == 1], minlength=N_NODES)

    # pack each (core, half) separately
    packs = {}
    Ghalf = 0
    for c in range(NCORES):
        for hh in range(2):
            n = np.nonzero((node_core == c) & (half == hh))[0]
            g, ng = _pack_groups(dA[n], dB[n])
            packs[(c, hh)] = (n, g)
            Ghalf = max(Ghalf, ng)
    Ghalf = -(-Ghalf // 4) * 4            # multiple of 4: NSUP even, pair
                                          # P/2 boundary on a super boundary
    G = 2 * Ghalf
    P = G // 2                            # pairs (128-node tiles)
    NSUP = G // 4
    SH = G * GSZ                          # rows per core shard
    HSH = SH // 2
    RT = NCORES * HSH                     # rows per half-table
    assert RT <= 32767, f"int16 overflow: {RT}"

    # node -> loc (row within core shard)
    loc = np.full(N_NODES, -1, dtype=np.int64)
    for c in range(NCORES):
        for hh in range(2):
            n, g = packs[(c, hh)]
            order = np.lexsort((n, g))
            n_s, g_s = n[order], g[order]
            slot = np.zeros(len(n), dtype=np.int64)
            _, starts = np.unique(g_s, return_index=True)
            for s0, s1 in zip(starts, list(starts[1:]) + [len(n)]):
                slot[s0:s1] = np.arange(s1 - s0)
            loc[n_s] = (hh * Ghalf + g_s) * GSZ + slot

    lochalf = loc % HSH                   # row within own (core, half) shard
    QSH = HSH // 2
    srow = ((lochalf >= QSH).astype(np.int64) * (RT // 2)
            + node_core * QSH + (lochalf % QSH))  # quarter-major half-table row

    # per-core edge layout
    e_core = node_core[dst]
    e_group = loc[dst] // GSZ
    e_dslot = loc[dst] % GSZ
    e_srow = srow[src]

    per_core = []
    for c in range(NCORES):
        m = np.nonzero(e_core == c)[0]
        g, h, sr, dslt = e_group[m], src_half[m], e_srow[m], e_dslot[m]
        order = np.lexsort((sr, h, g))
        g, h, sr, dslt = g[order], h[order], sr[order], dslt[order]
        key = g * 2 + h
        rank = np.arange(len(m)) - np.searchsorted(key, key, side="left")
        k = rank // 128
        p = rank % 128
        assert (k < CHUNKS_PER_HALF).all(), "cap exceeded"
        s = g // 4
        gi = g % 4
        c16 = gi * CHUNKS_PER_HALF + k        # chunk col within (super, half)
        j = c16 * 128 + p                     # slot within (super, half)
        t = s * 2 + h

        idx16 = np.zeros((2 * NSUP, 16, 128), dtype=np.int16)
        idx16[t, j % 16, j // 16] = sr.astype(np.int16)
        idx_all = np.tile(idx16, (1, 8, 1)).reshape(2 * NSUP, 128, 128)
        idx_2d = idx_all.transpose(1, 0, 2).reshape(128, 2 * NSUP * 128).copy()

        dstv = np.full((128, 2 * NSUP * 16), -1.0, dtype=np.float32)
        dstv[j % 128, t * 16 + c16] = dslt.astype(np.float32)

        # per-pair node scalars [128, P]
        nodes_c = np.nonzero(node_core == c)[0]
        lr = loc[nodes_c]
        ni_t = np.ones((128, P), dtype=np.float32)
        no_t = np.ones((128, P), dtype=np.float32)
        gid_t = np.full((128, P), -1, dtype=np.int64)
        pr = lr // 128
        pp = lr % 128
        ni_t[pp, pr] = 1.0 / np.sqrt(deg_in[nodes_c])
        no_t[pp, pr] = 1.0 / np.sqrt(deg_out[nodes_c])
        gid_t[pp, pr] = graph_ids[nodes_c] - c * GPC
        groh = np.zeros((128, P, GPC), dtype=np.float32)
        pg, prr = np.nonzero(gid_t >= 0)
        groh[pg, prr, gid_t[pg, prr]] = 1.0

        rc = (1.0 / np.maximum(
            np.bincount(graph_ids[nodes_c] - c * GPC, minlength=GPC), 1
        )).astype(np.float32).reshape(GPC, 1)

        per_core.append(dict(
            idx=idx_2d, dstv=_to_bf16(dstv), ni=ni_t, no=no_t,
            groh=_to_bf16(groh), rc=rc,
        ))

    # layer-0 half-tables: hn0 = feats * norm_out, bf16, in AllGather layout
    hn0 = feats * (1.0 / np.sqrt(deg_out))[:, None]
    t0 = np.zeros((2, RT, D), dtype=np.float32)
    nodes = np.nonzero(loc >= 0)[0]
    t0[half[nodes], srow[nodes]] = hn0[nodes]
    table0A = _to_bf16(t0[0])
    table0B = _to_bf16(t0[1])

    b_rep = np.broadcast_to(
        np.asarray(b, dtype=np.float32)[None, :, :], (128, N_LAYERS, D)
    ).copy()
    W_t = _to_bf16(np.ascontiguousarray(
        np.asarray(W, dtype=np.float32).transpose(1, 0, 2)))

    meta = dict(G=G, P=P, NSUP=NSUP, SH=SH, HSH=HSH, RT=RT)
    shared = dict(W=W_t, b_rep=b_rep, t0A=table0A, t0B=table0B)
    return per_core, shared, meta


# ------------------------------------------------------------- device build
def build(meta, rep=1, no_coll=False, mode="full", nq=NQ, pieces=PIECES,
          gbufs=4, obufs=12, pbufs=5, sbufs=8, half_mm=False, dbl_oh=False,
          spkt=False, agdelay=6):
    G, P, NSUP, SH, HSH, RT = (meta[k] for k in ("G", "P", "NSUP", "SH", "HSH", "RT"))
    CH = CHUNKS_PER_HALF
    f32 = mybir.dt.float32
    bf16 = mybir.dt.bfloat16

    nc = bacc.Bacc("TRN2", target_bir_lowering=False, debug=False,
                   num_devices=NCORES, dynamic_dma_scratch_size=16384,
                   num_swdge_queues=nq)

    idx_t = nc.dram_tensor("idx", [128, 2 * NSUP * 128], mybir.dt.int16, kind="ExternalInput")
    dstv_t = nc.dram_tensor("dstv", [128, 2 * NSUP * 16], bf16, kind="ExternalInput")
    ni_t = nc.dram_tensor("ni", [128, P], f32, kind="ExternalInput")
    no_t = nc.dram_tensor("no", [128, P], f32, kind="ExternalInput")
    groh_t = nc.dram_tensor("groh", [128, P, GPC], bf16, kind="ExternalInput")
    rc_t = nc.dram_tensor("rc", [GPC, 1], f32, kind="ExternalInput")
    t0A_t = nc.dram_tensor("t0A", [RT, D], bf16, kind="ExternalInput")
    t0B_t = nc.dram_tensor("t0B", [RT, D], bf16, kind="ExternalInput")
    W_t = nc.dram_tensor("W", [128, N_LAYERS, D], bf16, kind="ExternalInput")
    brep_t = nc.dram_tensor("b_rep", [128, N_LAYERS, D], f32, kind="ExternalInput")
    out_t = nc.dram_tensor("out", [GPC, D], f32, kind="ExternalOutput")

    # AllGather outputs for layers 1, 2: [half][RT, D]
    ag = [[nc.dram_tensor(f"ag{l}_{q}", [RT, D], bf16,
                          kind="Internal", addr_space="Shared") for q in range(2)]
          for l in range(1, N_LAYERS)]

    SCOL = (2048 // pieces) // 16          # idx cols per gather piece
    CPP = (CH * 4) // pieces               # chunks per gather piece

    with tile.TileContext(nc) as tc, ExitStack() as ctx:
        dram = ctx.enter_context(tc.tile_pool(name="dram", bufs=1, space="DRAM"))
        stat = ctx.enter_context(tc.tile_pool(name="stat", bufs=1))
        gpool = ctx.enter_context(tc.tile_pool(name="gath", bufs=gbufs))
        opool = ctx.enter_context(tc.tile_pool(name="oh", bufs=obufs))
        spool = ctx.enter_context(tc.tile_pool(name="sb", bufs=sbufs))
        ppool = ctx.enter_context(tc.tile_pool(name="agg_ps", bufs=pbufs, space="PSUM"))
        hpool = ctx.enter_context(tc.tile_pool(name="h_ps", bufs=2, space="PSUM"))
        plpool = ctx.enter_context(tc.tile_pool(name="pool_ps", bufs=1, space="PSUM"))

        hn_shard = dram.tile([SH, D], bf16)

        # ---- statics
        idx_sb = stat.tile([128, 2 * NSUP * 128], mybir.dt.int16)
        nc.sync.dma_start(idx_sb[:], idx_t.ap())
        dstv_sb = stat.tile([128, 2 * NSUP * 16], bf16)
        nc.sync.dma_start(dstv_sb[:], dstv_t.ap())
        W_sb = stat.tile([128, N_LAYERS, D], bf16)
        nc.sync.dma_start(W_sb[:], W_t.ap())
        brep_sb = stat.tile([128, N_LAYERS, D], f32)
        nc.sync.dma_start(brep_sb[:], brep_t.ap())
        groh_sb = stat.tile([128, P, GPC], bf16)
        nc.sync.dma_start(groh_sb[:], groh_t.ap())
        ni_sb = stat.tile([128, P], f32)
        nc.sync.dma_start(ni_sb[:], ni_t.ap())
        no_sb = stat.tile([128, P], f32)
        nc.sync.dma_start(no_sb[:], no_t.ap())
        rc_sb = stat.tile([GPC, 1], f32)
        nc.sync.dma_start(rc_sb[:], rc_t.ap())

        iota16 = stat.tile([128, GSZ], mybir.dt.int16)
        nc.gpsimd.iota(iota16[:], pattern=[[1, GSZ]], base=0, channel_multiplier=0)
        iota_f = stat.tile([128, GSZ], bf16)
        nc.vector.tensor_copy(iota_f[:], iota16[:])

        aggA_sb = stat.tile([128, P, 128], bf16)
        aggB_sb = stat.tile([128, P, 128], bf16)
        if mode == "compute_only":
            gstatA = stat.tile([128, 4 * CH, D], bf16)
            nc.vector.memset(gstatA[:], 0.25)
            gstatB = stat.tile([128, 4 * CH, D], bf16)
            nc.vector.memset(gstatB[:], 0.25)

        qctr = [0]

        def gather_half_fn(gt, src_ap, s, hh):
            base_col = (2 * s + hh) * 128
            for piece in range(pieces):
                nc.gpsimd.dma_gather(
                    out_ap=gt[:, piece * CPP:(piece + 1) * CPP, :],
                    in_ap=src_ap,
                    idxs_ap=idx_sb[:, base_col + piece * SCOL:
                                   base_col + (piece + 1) * SCOL],
                    num_idxs=2048 // pieces, num_idxs_reg=2048 // pieces,
                    elem_size=D, single_packet=spkt,
                    queue_num=qctr[0] % nq,
                )
                qctr[0] += 1

        def build_oh(s, hh, tag):
            oh = opool.tile([128, 4 * CH, GSZ], bf16, tag=tag)
            t16 = (2 * s + hh) * 16
            for _r in range(2 if dbl_oh else 1):
                nc.vector.tensor_tensor(
                    out=oh[:],
                    in0=iota_f[:].unsqueeze(1).broadcast_to([128, 4 * CH, GSZ]),
                    in1=dstv_sb[:, t16:t16 + 16]
                        .unsqueeze(2).broadcast_to([128, 4 * CH, GSZ]),
                    op=mybir.AluOpType.is_equal,
                )
            return oh

        def seg_matmuls(gt, oh, s, agg_dst):
            """8 matmuls per pair accumulating [f, dslot] into agg PSUM,
            then evict to agg_dst[:, pr, :] (bf16)."""
            for pi in range(2):
                pr = s * 2 + pi
                agg = ppool.tile([128, 128], f32, tag="agg")
                for gj in range(2):
                    gi = pi * 2 + gj
                    off = gj * GSZ
                    ks = range(0, CH, 2) if half_mm else range(CH)
                    last = list(ks)[-1]
                    for k in ks:
                        nc.tensor.matmul(
                            agg[:, off:off + GSZ],
                            gt[:, gi * CH + k, :],
                            oh[:, gi * CH + k, :],
                            start=(k == 0), stop=(k == last and gj == 1),
                            skip_group_check=True,
                        )
                nc.scalar.copy(agg_dst[:, pr, :], agg[:])

        gather_half = gather_half_fn if mode != "compute_only" else (lambda *a: None)
        if mode == "gather_only":
            acc = stat.tile([128, 1], f32)
            nc.vector.memset(acc[:], 0.0)
            dump = stat.tile([GPC, D], f32)
            nc.vector.memset(dump[:], 0.0)
            for _ in range(rep):
                for l in range(N_LAYERS):
                    for hh in range(2):
                        src_ap = (t0A_t.ap() if hh == 0 else t0B_t.ap())
                        for s in range(NSUP):
                            gt = gpool.tile([128, 4 * CH, D], bf16, tag="g")
                            gather_half_fn(gt, src_ap, s, hh)
                            for piece in range(pieces):
                                nc.vector.tensor_tensor(
                                    out=acc[:], in0=acc[:],
                                    in1=gt[:, piece * CPP, 0:1],
                                    op=mybir.AluOpType.add)
            nc.sync.dma_start(out_t.ap(), dump[:])
            rep = 0
        for _ in range(rep):
            pool_ps = plpool.tile([GPC, D], f32)

            for l in range(N_LAYERS):
                if l == 0:
                    srcA, srcB = t0A_t.ap(), t0B_t.ap()
                else:
                    srcA, srcB = ag[l - 1][0].ap(), ag[l - 1][1].ap()

                # layer 0 has both tables at launch: single-pass supers
                # (one PSUM group of 32 matmuls, one evict, one dense)
                single = (l == 0 and mode == "full")
                if not single:
                    # phase A: all supers' A-half work
                    for s in range(NSUP):
                        if mode == "compute_only":
                            gA = gstatA
                        else:
                            gA = gpool.tile([128, 4 * CH, D], bf16, tag="gA")
                            gather_half(gA, srcA, s, 0)
                        ohA = build_oh(s, 0, "ohA")
                        seg_matmuls(gA, ohA, s, aggA_sb)

                # phase B + dense + epilogue
                for s in range(NSUP):
                    if mode == "compute_only":
                        gB = gstatB
                    else:
                        gB = gpool.tile([128, 4 * CH, D], bf16, tag="gB")
                        gather_half(gB, srcB, s, 1)
                    ohB = build_oh(s, 1, "ohB")
                    if single:
                        gA = gpool.tile([128, 4 * CH, D], bf16, tag="gA")
                        gather_half(gA, srcA, s, 0)
                        ohA = build_oh(s, 0, "ohA")
                        for pi in range(2):
                            pr = s * 2 + pi
                            agg = ppool.tile([128, 128], f32, tag="agg")
                            for hi, (gt, oh) in enumerate(((gA, ohA), (gB, ohB))):
                                for gj in range(2):
                                    gi = pi * 2 + gj
                                    off = gj * GSZ
                                    for k in range(CH):
                                        nc.tensor.matmul(
                                            agg[:, off:off + GSZ],
                                            gt[:, gi * CH + k, :],
                                            oh[:, gi * CH + k, :],
                                            start=(hi == 0 and gj == 0 and k == 0),
                                            stop=(hi == 1 and gj == 1 and k == CH - 1),
                                            skip_group_check=True,
                                        )
                            nc.scalar.copy(aggB_sb[:, pr, :], agg[:])
                    else:
                        seg_matmuls(gB, ohB, s, aggB_sb)
                    for pi in range(2):
                        pr = s * 2 + pi
                        hps = hpool.tile([128, D], f32, tag="hps")
                        if single:
                            nc.tensor.matmul(hps[:], aggB_sb[:, pr, :],
                                             W_sb[:, l, :], start=True, stop=True,
                                             skip_group_check=True)
                        else:
                            nc.tensor.matmul(hps[:], aggA_sb[:, pr, :],
                                             W_sb[:, l, :], start=True, stop=False,
                                             skip_group_check=True)
                            nc.tensor.matmul(hps[:], aggB_sb[:, pr, :],
                                             W_sb[:, l, :], start=False, stop=True,
                                             skip_group_check=True)
                        t_sb = spool.tile([128, D], f32, tag="tsb")
                        nc.vector.scalar_tensor_tensor(
                            out=t_sb[:], in0=hps[:], scalar=ni_sb[:, pr:pr + 1],
                            in1=brep_sb[:, l, :],
                            op0=mybir.AluOpType.mult, op1=mybir.AluOpType.add,
                        )
                        if l < N_LAYERS - 1:
                            hn = spool.tile([128, D], bf16, tag="hn")
                            nc.vector.tensor_scalar(
                                out=hn[:], in0=t_sb[:],
                                scalar1=0.0, scalar2=no_sb[:, pr:pr + 1],
                                op0=mybir.AluOpType.max, op1=mybir.AluOpType.mult,
                            )
                            nc.sync.dma_start(
                                hn_shard[pr * 128:(pr + 1) * 128, :], hn[:])
                        else:
                            h_sb = spool.tile([128, D], bf16, tag="hsb")
                            nc.vector.tensor_scalar_max(h_sb[:], t_sb[:], 0.0)
                            nc.tensor.matmul(
                                pool_ps[:], groh_sb[:, pr, :], h_sb[:],
                                start=(pr == 0), stop=(pr == P - 1),
                                skip_group_check=True,
                            )
                    # fire next layer's AllGathers quarter-by-quarter as
                    # soon as each input slice is fully written
                    if l < N_LAYERS - 1 and not no_coll and mode == "full":
                        QSH = HSH // 2
                        for qi in range(4):           # quarter qi = rows
                            # [qi*QSH, (qi+1)*QSH) -> ag[half][q2 slice]
                            # issue `agdelay` supers after input readiness so
                            # the collective's sem wait does not stall the
                            # Pool SEQ between gather issues
                            need_pairs = ((qi + 1) * QSH + 127) // 128
                            fire_s = min((need_pairs + 1) // 2 - 1 + agdelay,
                                         NSUP - 1)
                            if s == fire_s:
                                hhq, q2 = qi // 2, qi % 2
                                nc.gpsimd.collective_compute(
                                    "AllGather", mybir.AluOpType.bypass,
                                    replica_groups=[list(range(NCORES))],
                                    ins=[hn_shard[qi * QSH:(qi + 1) * QSH, :].opt()],
                                    outs=[ag[l][hhq].ap()
                                          [q2 * (RT // 2):(q2 + 1) * (RT // 2), :].opt()],
                                )

            pool_sb = spool.tile([GPC, D], f32, tag="poolsb")
            nc.vector.tensor_scalar_mul(pool_sb[:], pool_ps[:], rc_sb[:])
            nc.sync.dma_start(out_t.ap(), pool_sb[:])

    nc.compile()
    return nc


def make_in_maps(per_core, shared):
    in_maps = []
    for c in range(NCORES):
        pc = per_core[c]
        in_maps.append({
            "idx": pc["idx"], "dstv": pc["dstv"], "ni": pc["ni"],
            "no": pc["no"], "groh": pc["groh"], "rc": pc["rc"],
            "t0A": shared["t0A"], "t0B": shared["t0B"],
            "W": shared["W"], "b_rep": shared["b_rep"],
        })
    return in_maps


def kernel(**inputs) -> np.ndarray:
    per_core, shared, meta = preprocess(**inputs)
    nc = build(meta, rep=1)
    in_maps = make_in_maps(per_core, shared)
    res = run_bass_kernel_spmd(nc, in_maps, core_ids=list(range(NCORES)))
    return np.concatenate([res.results[c]["out"] for c in range(NCORES)], axis=0)


# revision 19
# speedup vs baseline: 8.2346x; 1.2803x over previous
"""BRPConvEmbedding (3-layer GraphConv + AvgPool readout) on 8 Trainium2 cores.

Sharding: graphs split contiguously across cores (32 graphs/core); each core
owns its graphs' nodes, so pooling is core-local and the output is a concat.
Within a core, nodes are pre-committed to two halves (A/B) and each half is
packed into dst-groups of <=64 nodes whose in-degree per source-half is capped
at 512 (4 chunks of 128 edge slots), giving a uniform SPMD layout.

Per layer the full node-feature table lives in two bf16 halves (all cores'
A-rows / B-rows), each built by two quarter-AllGathers that write disjoint
slices of one buffer and fire as their input rows complete; the collective
instructions are issued `agdelay` supers late so their semaphore waits do not
stall the Pool SEQ between dma_gather issues (collective_compute is
Pool-engine-only). Layer 0's halves are host-precomputed (feats * norm_out)
inputs, so layer 0 needs no collective and runs single-pass supers. SpMM:
per-edge rows are fetched with SWDGE dma_gather (int16 indices, 4 rotated
512-row pieces over 4 queues), the per-edge one-hot is built on the VectorE,
and the segment-sum runs on the TensorE via gathered-chunk x one-hot matmuls
in bf16 with fp32 PSUM accumulation. For layers 1-2 the A-half runs for all
supers first (so the B-half AllGather overlaps compute); each pair's A/B
partial aggregates are evicted to SBUF and combined by two accumulating
W-matmuls, then the epilogue (x norm_in, +b, relu, x norm_out) writes bf16
rows for the next AllGather.
"""
import numpy as np
from contextlib import ExitStack

import concourse.bacc as bacc
import concourse.mybir as mybir
from concourse import tile
from concourse.bass_utils import run_bass_kernel_spmd

N_NODES = 50000
N_EDGES = 800000
D = 128
N_LAYERS = 3
N_GRAPHS = 256
NCORES = 8
GSZ = 64                  # dst nodes per group
CHUNKS_PER_HALF = 4       # 4*128 = 512 edge-slot cap per (group, src-half)
CAP = CHUNKS_PER_HALF * 128
GPC = N_GRAPHS // NCORES  # graphs per core
NQ = 4                    # SWDGE queues
PIECES = 4                # gather instructions per (super, half)


# ----------------------------------------------------------------- host prep
def _pack_groups(dA, dB):
    """Greedy bin-packing: nodes (rows of dA/dB) into groups of <= GSZ nodes
    with sum(dA) <= CAP and sum(dB) <= CAP. Returns group ids."""
    n = len(dA)
    order = np.argsort(-np.maximum(dA, dB), kind="stable")
    gids = np.full(n, -1, dtype=np.int64)
    usedA, usedB, usedN = [], [], []
    for i in order:
        a, b = dA[i], dB[i]
        best, best_fit = -1, -1.0
        for g in range(len(usedA)):
            if usedN[g] < GSZ and usedA[g] + a <= CAP and usedB[g] + b <= CAP:
                fit = max((usedA[g] + a) / CAP, (usedB[g] + b) / CAP)
                if fit > best_fit:
                    best, best_fit = g, fit
        if best < 0:
            usedA.append(0), usedB.append(0), usedN.append(0)
            best = len(usedA) - 1
        gids[i] = best
        usedA[best] += a
        usedB[best] += b
        usedN[best] += 1
    return gids, len(usedA)


def _to_bf16(x):
    import jax.numpy as jnp
    return np.asarray(jnp.asarray(np.asarray(x, np.float32), dtype=jnp.bfloat16))


def preprocess(feats, W, b, src, dst, graph_ids):
    src = np.asarray(src).astype(np.int64)
    dst = np.asarray(dst).astype(np.int64)
    graph_ids = np.asarray(graph_ids).astype(np.int64)
    feats = np.asarray(feats, dtype=np.float32)

    deg_out = np.maximum(np.bincount(src, minlength=N_NODES), 1).astype(np.float32)
    deg_in = np.maximum(np.bincount(dst, minlength=N_NODES), 1).astype(np.float32)
    node_core = graph_ids // GPC

    # pre-commit each node to half A(0)/B(1): per core, alternate over nodes
    # sorted by out-degree so both the source split and node counts balance
    half = np.zeros(N_NODES, dtype=np.int64)
    for c in range(NCORES):
        n = np.nonzero(node_core == c)[0]
        order = n[np.argsort(-deg_out[n], kind="stable")]
        half[order[1::2]] = 1

    src_half = half[src]
    dA = np.bincount(dst[src_half == 0], minlength=N_NODES)
    dB = np.bincount(dst[src_half == 1], minlength=N_NODES)

    # pack each (core, half) separately
    packs = {}
    Ghalf = 0
    for c in range(NCORES):
        for hh in range(2):
            n = np.nonzero((node_core == c) & (half == hh))[0]
            g, ng = _pack_groups(dA[n], dB[n])
            packs[(c, hh)] = (n, g)
            Ghalf = max(Ghalf, ng)
    Ghalf = -(-Ghalf // 4) * 4            # multiple of 4: NSUP even, pair
                                          # P/2 boundary on a super boundary
    G = 2 * Ghalf
    P = G // 2                            # pairs (128-node tiles)
    NSUP = G // 4
    SH = G * GSZ                          # rows per core shard
    HSH = SH // 2
    RT = NCORES * HSH                     # rows per half-table
    assert RT <= 32767, f"int16 overflow: {RT}"

    # node -> loc (row within core shard)
    loc = np.full(N_NODES, -1, dtype=np.int64)
    for c in range(NCORES):
        for hh in range(2):
            n, g = packs[(c, hh)]
            order = np.lexsort((n, g))
            n_s, g_s = n[order], g[order]
            slot = np.zeros(len(n), dtype=np.int64)
            _, starts = np.unique(g_s, return_index=True)
            for s0, s1 in zip(starts, list(starts[1:]) + [len(n)]):
                slot[s0:s1] = np.arange(s1 - s0)
            loc[n_s] = (hh * Ghalf + g_s) * GSZ + slot

    lochalf = loc % HSH                   # row within own (core, half) shard
    QSH = HSH // 2
    srow = ((lochalf >= QSH).astype(np.int64) * (RT // 2)
            + node_core * QSH + (lochalf % QSH))  # quarter-major half-table row

    # per-core edge layout
    e_core = node_core[dst]
    e_group = loc[dst] // GSZ
    e_dslot = loc[dst] % GSZ
    e_srow = srow[src]

    per_core = []
    for c in range(NCORES):
        m = np.nonzero(e_core == c)[0]
        g, h, sr, dslt = e_group[m], src_half[m], e_srow[m], e_dslot[m]
        order = np.lexsort((sr, h, g))
        g, h, sr, dslt = g[order], h[order], sr[order], dslt[order]
        key = g * 2 + h
        rank = np.arange(len(m)) - np.searchsorted(key, key, side="left")
        k = rank // 128
        p = rank % 128
        assert (k < CHUNKS_PER_HALF).all(), "cap exceeded"
        s = g // 4
        gi = g % 4
        c16 = gi * CHUNKS_PER_HALF + k        # chunk col within (super, half)
        j = c16 * 128 + p                     # slot within (super, half)
        t = s * 2 + h

        idx16 = np.zeros((2 * NSUP, 16, 128), dtype=np.int16)
        idx16[t, j % 16, j // 16] = sr.astype(np.int16)
        idx_all = np.tile(idx16, (1, 8, 1)).reshape(2 * NSUP, 128, 128)
        idx_2d = idx_all.transpose(1, 0, 2).reshape(128, 2 * NSUP * 128).copy()

        dstv = np.full((128, 2 * NSUP * 16), -1.0, dtype=np.float32)
        dstv[j % 128, t * 16 + c16] = dslt.astype(np.float32)

        # per-pair node scalars [128, P]
        nodes_c = np.nonzero(node_core == c)[0]
        lr = loc[nodes_c]
        ni_t = np.ones((128, P), dtype=np.float32)
        no_t = np.ones((128, P), dtype=np.float32)
        gid_t = np.full((128, P), -1, dtype=np.int64)
        pr = lr // 128
        pp = lr % 128
        ni_t[pp, pr] = 1.0 / np.sqrt(deg_in[nodes_c])
        no_t[pp, pr] = 1.0 / np.sqrt(deg_out[nodes_c])
        gid_t[pp, pr] = graph_ids[nodes_c] - c * GPC
        groh = np.zeros((128, P, GPC), dtype=np.float32)
        pg, prr = np.nonzero(gid_t >= 0)
        groh[pg, prr, gid_t[pg, prr]] = 1.0

        rc = (1.0 / np.maximum(
            np.bincount(graph_ids[nodes_c] - c * GPC, minlength=GPC), 1
        )).astype(np.float32).reshape(GPC, 1)

        per_core.append(dict(
            idx=idx_2d, dstv=_to_bf16(dstv), ni=ni_t, no=no_t,
            groh=_to_bf16(groh), rc=rc,
        ))

    # layer-0 half-tables: hn0 = feats * norm_out, bf16, in AllGather layout
    hn0 = feats * (1.0 / np.sqrt(deg_out))[:, None]
    t0 = np.zeros((2, RT, D), dtype=np.float32)
    nodes = np.nonzero(loc >= 0)[0]
    t0[half[nodes], srow[nodes]] = hn0[nodes]
    table0A = _to_bf16(t0[0])
    table0B = _to_bf16(t0[1])

    b_rep = np.broadcast_to(
        np.asarray(b, dtype=np.float32)[None, :, :], (128, N_LAYERS, D)
    ).copy()
    W_t = _to_bf16(np.ascontiguousarray(
        np.asarray(W, dtype=np.float32).transpose(1, 0, 2)))

    meta = dict(G=G, P=P, NSUP=NSUP, SH=SH, HSH=HSH, RT=RT)
    shared = dict(W=W_t, b_rep=b_rep, t0A=table0A, t0B=table0B)
    return per_core, shared, meta


# ------------------------------------------------------------- device build
def build(meta, rep=1, no_coll=False, mode="full", nq=NQ, pieces=PIECES,
          gbufs=4, obufs=12, pbufs=5, sbufs=8, half_mm=False, dbl_oh=False,
          spkt=False, agdelay=6, scratch=16384):
    G, P, NSUP, SH, HSH, RT = (meta[k] for k in ("G", "P", "NSUP", "SH", "HSH", "RT"))
    CH = CHUNKS_PER_HALF
    f32 = mybir.dt.float32
    bf16 = mybir.dt.bfloat16

    nc = bacc.Bacc("TRN2", target_bir_lowering=False, debug=False,
                   num_devices=NCORES, dynamic_dma_scratch_size=scratch,
                   num_swdge_queues=nq)

    idx_t = nc.dram_tensor("idx", [128, 2 * NSUP * 128], mybir.dt.int16, kind="ExternalInput")
    dstv_t = nc.dram_tensor("dstv", [128, 2 * NSUP * 16], bf16, kind="ExternalInput")
    ni_t = nc.dram_tensor("ni", [128, P], f32, kind="ExternalInput")
    no_t = nc.dram_tensor("no", [128, P], f32, kind="ExternalInput")
    groh_t = nc.dram_tensor("groh", [128, P, GPC], bf16, kind="ExternalInput")
    rc_t = nc.dram_tensor("rc", [GPC, 1], f32, kind="ExternalInput")
    t0A_t = nc.dram_tensor("t0A", [RT, D], bf16, kind="ExternalInput")
    t0B_t = nc.dram_tensor("t0B", [RT, D], bf16, kind="ExternalInput")
    W_t = nc.dram_tensor("W", [128, N_LAYERS, D], bf16, kind="ExternalInput")
    brep_t = nc.dram_tensor("b_rep", [128, N_LAYERS, D], f32, kind="ExternalInput")
    out_t = nc.dram_tensor("out", [GPC, D], f32, kind="ExternalOutput")

    # AllGather outputs for layers 1, 2: [half][RT, D]
    ag = [[nc.dram_tensor(f"ag{l}_{q}", [RT, D], bf16,
                          kind="Internal", addr_space="Shared") for q in range(2)]
          for l in range(1, N_LAYERS)]

    SCOL = (2048 // pieces) // 16          # idx cols per gather piece
    CPP = (CH * 4) // pieces               # chunks per gather piece

    with tile.TileContext(nc) as tc, ExitStack() as ctx:
        dram = ctx.enter_context(tc.tile_pool(name="dram", bufs=1, space="DRAM"))
        stat = ctx.enter_context(tc.tile_pool(name="stat", bufs=1))
        gpool = ctx.enter_context(tc.tile_pool(name="gath", bufs=gbufs))
        opool = ctx.enter_context(tc.tile_pool(name="oh", bufs=obufs))
        spool = ctx.enter_context(tc.tile_pool(name="sb", bufs=sbufs))
        ppool = ctx.enter_context(tc.tile_pool(name="agg_ps", bufs=pbufs, space="PSUM"))
        hpool = ctx.enter_context(tc.tile_pool(name="h_ps", bufs=2, space="PSUM"))
        plpool = ctx.enter_context(tc.tile_pool(name="pool_ps", bufs=1, space="PSUM"))

        hn_shard = dram.tile([SH, D], bf16)

        # ---- statics
        idx_sb = stat.tile([128, 2 * NSUP * 128], mybir.dt.int16)
        nc.sync.dma_start(idx_sb[:], idx_t.ap())
        dstv_sb = stat.tile([128, 2 * NSUP * 16], bf16)
        nc.sync.dma_start(dstv_sb[:], dstv_t.ap())
        W_sb = stat.tile([128, N_LAYERS, D], bf16)
        nc.sync.dma_start(W_sb[:], W_t.ap())
        brep_sb = stat.tile([128, N_LAYERS, D], f32)
        nc.sync.dma_start(brep_sb[:], brep_t.ap())
        groh_sb = stat.tile([128, P, GPC], bf16)
        nc.sync.dma_start(groh_sb[:], groh_t.ap())
        ni_sb = stat.tile([128, P], f32)
        nc.sync.dma_start(ni_sb[:], ni_t.ap())
        no_sb = stat.tile([128, P], f32)
        nc.sync.dma_start(no_sb[:], no_t.ap())
        rc_sb = stat.tile([GPC, 1], f32)
        nc.sync.dma_start(rc_sb[:], rc_t.ap())

        iota16 = stat.tile([128, GSZ], mybir.dt.int16)
        nc.gpsimd.iota(iota16[:], pattern=[[1, GSZ]], base=0, channel_multiplier=0)
        iota_f = stat.tile([128, GSZ], bf16)
        nc.vector.tensor_copy(iota_f[:], iota16[:])

        aggA_sb = stat.tile([128, P, 128], bf16)
        aggB_sb = stat.tile([128, P, 128], bf16)
        if mode == "compute_only":
            gstatA = stat.tile([128, 4 * CH, D], bf16)
            nc.vector.memset(gstatA[:], 0.25)
            gstatB = stat.tile([128, 4 * CH, D], bf16)
            nc.vector.memset(gstatB[:], 0.25)

        qctr = [0]

        def gather_half_fn(gt, src_ap, s, hh):
            base_col = (2 * s + hh) * 128
            for piece in range(pieces):
                nc.gpsimd.dma_gather(
                    out_ap=gt[:, piece * CPP:(piece + 1) * CPP, :],
                    in_ap=src_ap,
                    idxs_ap=idx_sb[:, base_col + piece * SCOL:
                                   base_col + (piece + 1) * SCOL],
                    num_idxs=2048 // pieces, num_idxs_reg=2048 // pieces,
                    elem_size=D, single_packet=spkt,
                    queue_num=qctr[0] % nq,
                )
                qctr[0] += 1

        def build_oh(s, hh, tag):
            oh = opool.tile([128, 4 * CH, GSZ], bf16, tag=tag)
            t16 = (2 * s + hh) * 16
            for _r in range(2 if dbl_oh else 1):
                nc.vector.tensor_tensor(
                    out=oh[:],
                    in0=iota_f[:].unsqueeze(1).broadcast_to([128, 4 * CH, GSZ]),
                    in1=dstv_sb[:, t16:t16 + 16]
                        .unsqueeze(2).broadcast_to([128, 4 * CH, GSZ]),
                    op=mybir.AluOpType.is_equal,
                )
            return oh

        def seg_matmuls(gt, oh, s, agg_dst):
            """8 matmuls per pair accumulating [f, dslot] into agg PSUM,
            then evict to agg_dst[:, pr, :] (bf16)."""
            for pi in range(2):
                pr = s * 2 + pi
                agg = ppool.tile([128, 128], f32, tag="agg")
                for gj in range(2):
                    gi = pi * 2 + gj
                    off = gj * GSZ
                    ks = range(0, CH, 2) if half_mm else range(CH)
                    last = list(ks)[-1]
                    for k in ks:
                        nc.tensor.matmul(
                            agg[:, off:off + GSZ],
                            gt[:, gi * CH + k, :],
                            oh[:, gi * CH + k, :],
                            start=(k == 0), stop=(k == last and gj == 1),
                            skip_group_check=True,
                        )
                nc.scalar.copy(agg_dst[:, pr, :], agg[:])

        gather_half = gather_half_fn if mode != "compute_only" else (lambda *a: None)
        if mode == "gather_only":
            acc = stat.tile([128, 1], f32)
            nc.vector.memset(acc[:], 0.0)
            dump = stat.tile([GPC, D], f32)
            nc.vector.memset(dump[:], 0.0)
            for _ in range(rep):
                for l in range(N_LAYERS):
                    for hh in range(2):
                        src_ap = (t0A_t.ap() if hh == 0 else t0B_t.ap())
                        for s in range(NSUP):
                            gt = gpool.tile([128, 4 * CH, D], bf16, tag="g")
                            gather_half_fn(gt, src_ap, s, hh)
                            for piece in range(pieces):
                                nc.vector.tensor_tensor(
                                    out=acc[:], in0=acc[:],
                                    in1=gt[:, piece * CPP, 0:1],
                                    op=mybir.AluOpType.add)
            nc.sync.dma_start(out_t.ap(), dump[:])
            rep = 0
        for _ in range(rep):
            pool_ps = plpool.tile([GPC, D], f32)

            for l in range(N_LAYERS):
                if l == 0:
                    srcA, srcB = t0A_t.ap(), t0B_t.ap()
                else:
                    srcA, srcB = ag[l - 1][0].ap(), ag[l - 1][1].ap()

                # layer 0 has both tables at launch: single-pass supers
                # (one PSUM group of 32 matmuls, one evict, one dense)
                single = (l == 0 and mode == "full")
                if not single:
                    # phase A: all supers' A-half work
                    for s in range(NSUP):
                        if mode == "compute_only":
                            gA = gstatA
                        else:
                            gA = gpool.tile([128, 4 * CH, D], bf16, tag="gA")
                            gather_half(gA, srcA, s, 0)
                        ohA = build_oh(s, 0, "ohA")
                        seg_matmuls(gA, ohA, s, aggA_sb)

                # phase B + dense + epilogue
                for s in range(NSUP):
                    if mode == "compute_only":
                        gB = gstatB
                    else:
                        gB = gpool.tile([128, 4 * CH, D], bf16, tag="gB")
                        gather_half(gB, srcB, s, 1)
                    ohB = build_oh(s, 1, "ohB")
                    if single:
                        gA = gpool.tile([128, 4 * CH, D], bf16, tag="gA")
                        gather_half(gA, srcA, s, 0)
                        ohA = build_oh(s, 0, "ohA")
                        for pi in range(2):
                            pr = s * 2 + pi
                            agg = ppool.tile([128, 128], f32, tag="agg")
                            for hi, (gt, oh) in enumerate(((gA, ohA), (gB, ohB))):
                                for gj in range(2):
                                    gi = pi * 2 + gj
                                    off = gj * GSZ
                                    for k in range(CH):
                                        nc.tensor.matmul(
                                            agg[:, off:off + GSZ],
                                            gt[:, gi * CH + k, :],
                                            oh[:, gi * CH + k, :],
                                            start=(hi == 0 and gj == 0 and k == 0),
                                            stop=(hi == 1 and gj == 1 and k == CH - 1),
                                            skip_group_check=True,
                                        )
                            nc.scalar.copy(aggB_sb[:, pr, :], agg[:])
                    else:
                        seg_matmuls(gB, ohB, s, aggB_sb)
                    for pi in range(2):
                        pr = s * 2 + pi
                        hps = hpool.tile([128, D], f32, tag="hps")
                        if single:
                            nc.tensor.matmul(hps[:], aggB_sb[:, pr, :],
                                             W_sb[:, l, :], start=True, stop=True,
                                             skip_group_check=True)
                        else:
                            nc.tensor.matmul(hps[:], aggA_sb[:, pr, :],
                                             W_sb[:, l, :], start=True, stop=False,
                                             skip_group_check=True)
                            nc.tensor.matmul(hps[:], aggB_sb[:, pr, :],
                                             W_sb[:, l, :], start=False, stop=True,
                                             skip_group_check=True)
                        t_sb = spool.tile([128, D], f32, tag="tsb")
                        nc.vector.scalar_tensor_tensor(
                            out=t_sb[:], in0=hps[:], scalar=ni_sb[:, pr:pr + 1],
                            in1=brep_sb[:, l, :],
                            op0=mybir.AluOpType.mult, op1=mybir.AluOpType.add,
                        )
                        if l < N_LAYERS - 1:
                            hn = spool.tile([128, D], bf16, tag="hn")
                            nc.vector.tensor_scalar(
                                out=hn[:], in0=t_sb[:],
                                scalar1=0.0, scalar2=no_sb[:, pr:pr + 1],
                                op0=mybir.AluOpType.max, op1=mybir.AluOpType.mult,
                            )
                            nc.sync.dma_start(
                                hn_shard[pr * 128:(pr + 1) * 128, :], hn[:])
                        else:
                            h_sb = spool.tile([128, D], bf16, tag="hsb")
                            nc.vector.tensor_scalar_max(h_sb[:], t_sb[:], 0.0)
                            nc.tensor.matmul(
                                pool_ps[:], groh_sb[:, pr, :], h_sb[:],
                                start=(pr == 0), stop=(pr == P - 1),
                                skip_group_check=True,
                            )
                    # fire next layer's AllGathers quarter-by-quarter as
                    # soon as each input slice is fully written
                    if l < N_LAYERS - 1 and not no_coll and mode == "full":
                        QSH = HSH // 2
                        for qi in range(4):           # quarter qi = rows
                            # [qi*QSH, (qi+1)*QSH) -> ag[half][q2 slice]
                            # issue `agdelay` supers after input readiness so
                            # the collective's sem wait does not stall the
                            # Pool SEQ between gather issues
                            need_pairs = ((qi + 1) * QSH + 127) // 128
                            fire_s = min((need_pairs + 1) // 2 - 1 + agdelay,
                                         NSUP - 1)
                            if s == fire_s:
                                hhq, q2 = qi // 2, qi % 2
                                nc.gpsimd.collective_compute(
                                    "AllGather", mybir.AluOpType.bypass,
                                    replica_groups=[list(range(NCORES))],
                                    ins=[hn_shard[qi * QSH:(qi + 1) * QSH, :].opt()],
                                    outs=[ag[l][hhq].ap()
                                          [q2 * (RT // 2):(q2 + 1) * (RT // 2), :].opt()],
                                )

            pool_sb = spool.tile([GPC, D], f32, tag="poolsb")
            nc.vector.tensor_scalar_mul(pool_sb[:], pool_ps[:], rc_sb[:])
            nc.sync.dma_start(out_t.ap(), pool_sb[:])

    nc.compile()
    return nc


def make_in_maps(per_core, shared):
    in_maps = []
    for c in range(NCORES):
        pc = per_core[c]
        in_maps.append({
            "idx": pc["idx"], "dstv": pc["dstv"], "ni": pc["ni"],
            "no": pc["no"], "groh": pc["groh"], "rc": pc["rc"],
            "t0A": shared["t0A"], "t0B": shared["t0B"],
            "W": shared["W"], "b_rep": shared["b_rep"],
        })
    return in_maps


def kernel(**inputs) -> np.ndarray:
    per_core, shared, meta = preprocess(**inputs)
    nc = build(meta, rep=1)
    in_maps = make_in_maps(per_core, shared)
    res = run_bass_kernel_spmd(nc, in_maps, core_ids=list(range(NCORES)))
    return np.concatenate([res.results[c]["out"] for c in range(NCORES)], axis=0)
